# revision 32
# baseline (speedup 1.0000x reference)
"""Trainium2 Bass kernel for nn_HandIntersectionLoss.

Strategy
--------
Pure data parallel over batch: 64 batches -> 8 cores x 8 local batches.

Wall-clock per call is dominated by the axon tunnel, so the host ships
only the gathered hand points (u24-packed, ~40KB/core) and the device
derives all per-(batch,face) matmul constants itself:

  phase 0 (device):
    - one-hot face matrices from f32 face indices (K=1 broadcast matmul
      + is_equal against shipped iota columns)
    - triangle corners A,B,C per (batch,dir) via 2-chunk accumulated
      gather matmuls:  corners[3,500] = pts[128,3]^T @ onehot[128,500]
    - edges E1=B-A, E2=C-A; normal n = E1 x E2 via permutation-matmul
      rotations (engines cannot read partition offsets != 0)
    - dots |A|^2,.., 2A.B,.., 2A.n via ones/twos-column reduce matmuls
    - constants assembled into a persistent `staged` SBUF tile
      ([65,7,512]: 4 rows per (batch,dir) + shared coefficient row)
      via SBUF->SBUF DMAs (the only legal cross-partition mover)

  phase 1 (device): the proven compute loop. Per 128-point block:
    K=5 matmuls against staged constants produce la^2,lb^2,lc^2,
    2ab,2bc,2ca, 2det for [128 points x 500 faces]; per-element chain
    (denominator + range-reduced atan2) on DVE/ACT:

      atan2(det, den) = 2*atan(det / (rho + |den|))            (den >= 0)
                      = sign(det)*pi - 2*atan(det/(rho+|den|)) (den < 0)
      rho = sqrt(det^2 + den^2 + 1e-20)

    inside(p) <=> sum_f half > pi/2.  Min-distance via the same matmul
    trick against derived vert constants (mrhs) + free-dim min-reduce.
    Scalar-engine table sets force the two-pass structure (sqrt vs
    arctan live in different ACT table sets), staged in super-groups.

The jitted shard_map callable is cached across kernel() calls so repeat
calls skip jax retrace/XLA recompile entirely.

Wire format (wall clock = tunnel RTT ~40ms + payload at ~55-60MB/s, so
payload bytes rule everything): points ship as u24 fixed point over
[-8, 8) (abs step 9.5e-7; loss rel-err 5.2e-3, a plateau set by one
zero-margin inside/outside flip — f16 at 2.6e-2 breaches the 2e-2 gate,
u16 at 1.8e-2 has no margin, u21..u24 are all on the plateau).  Faces
ship as u8 (indices < 251).  Everything packs into ONE u8 array
[1024, 312] uploaded direct 8-way sharded ([128, 312] per core): one
command beats three by ~1ms, and direct sharding beats the old
dev0+reshard hop by ~1.3ms.  Byte planes are PLANAR (b0|b1|b2|faces),
which lets the relay's zstd shrink the low-entropy gaussian b2 plane:
319,488B raw -> ~240,500B on the wire when the compressor keeps up.
Device decodes pts = (b0 + b1*256 + b2*65536) * 16/(2^24-1) - 8 in
three engine ops.

Group semantics (raw, no halving on device):
  g0..2: xyz=A|B|C,       c3=|A|^2..,  w=1
  g3..5: xyz=(A+B)..raw,  c3=2A.B..,   w=2   -> col = 2*(A-p).(B-p)
  g6:    xyz=n raw,       c3=2*A.n,    w=0   -> col = 2*det
pass_a compensates with x0.5 folded into existing scalar_tensor_tensor.
"""
import sys
import numpy as np

sys.path.insert(0, '/opt/trn_rl_repo')

B, V_FULL, V_HAND, V_LOOP, N_FACES = 64, 6890, 250, 20, 500
P = V_HAND + 1          # 251 points/verts per hand (incl. lid)
PPAD = 256
NCORES = 8
NB = B // NCORES        # local batches per core
NBD = NB * 2            # (batch, dir) pairs per core
NBLK = NBD * 2          # blocks per core: x2 point-chunks of 128
SUPER = 8               # blocks per two-pass super-group
F = N_FACES
HALF_PI = float(np.pi / 2)

_compiled = None
SKIP_P1 = False
_runner = None
last_exec_time_ns = None

# u24 fixed point over [-8, 8): x -> round((x+8) * (2^24-1)/16)
Q_SCALE = (2.0 ** 24 - 1) / 16.0
DQ_SCALE = 16.0 / (2.0 ** 24 - 1)


# --------------------------------------------------------------------------
# host prep: index gathers + u24 encode (all heavy constant math on-device)
# --------------------------------------------------------------------------

# preallocated per-call buffers (pad columns written once; concat layouts
# built directly to skip per-core copies).  Pad points sit at 7.5 — far
# outside the unit-scale hand cloud (winding ~0) yet clear of the u24
# top end (8.0 would round to 2^24 in f32 and wrap to -8 in the byte
# split).
#
# Everything dynamic ships in ONE u8 array [1024, 312] (sharded to
# [128, 312] per core) — one command beats three by ~1ms of per-command
# tunnel overhead.  Byte PLANES are kept separate (planar, not
# interleaved triplets): the tunnel zstd-compresses the request, and the
# b2 plane (gaussian high byte, ~5.3 bits entropy) only compresses when
# contiguous.  Unit-stride planes also read faster on the DVE.
#   bytes [:,   0: 96] = b0 (low)   of u24 = round((x+8) * (2^24-1)/16)
#   bytes [:,  96:192] = b1 (mid)
#   bytes [:, 192:288] = b2 (high)
#   bytes [:, 288:312] = faces u8, row-major: buf[p, 288+j] = flat[p*24+j]
_pts_host = np.full((B, 2, PPAD, 3), 7.5, np.float32)
_ptsq = np.empty((NCORES, 128, 2, NBD, 3), np.float32)
_buf_concat = np.empty((NCORES * 128, 312), np.uint8)
_faces_concat = np.zeros((NCORES, 2, 3, 512), np.uint8)
_cst_concat = np.zeros((NCORES * 128, 8), np.float32)
for _c in range(NCORES):
    _cs = _cst_concat[_c * 128:(_c + 1) * 128]
    _cs[:, 0] = np.arange(128, dtype=np.float32)
    _cs[:, 1] = np.arange(128, 256, dtype=np.float32)
    for _m in range(3):
        _cs[(_m + 1) % 3, 2 + _m] = 1.0    # P1 (rot1)
        _cs[(_m + 2) % 3, 5 + _m] = 1.0    # P2 (rot2)
_extra_concat = np.ascontiguousarray(
    np.broadcast_to(np.arange(PPAD, dtype=np.float32), (NCORES, PPAD)))


def _prep(inputs):
    verts = np.asarray(inputs['verts_batch'], dtype=np.float32)
    hi = [np.asarray(inputs['hand_verts_inds_left']),
          np.asarray(inputs['hand_verts_inds_right'])]
    li = [np.asarray(inputs['hand_loop_verts_inds_left']),
          np.asarray(inputs['hand_loop_verts_inds_right'])]

    # pad stays 8.0 from init (pad cols never overwritten)
    for d in range(2):
        _pts_host[:, d, :V_HAND] = verts[:, hi[d]]
        _pts_host[:, d, V_HAND] = verts[:, li[d]].mean(axis=1,
                                                       dtype=np.float32)

    # [core, 128, 2kk, bd, 3] gather layout in one strided copy, then
    # u24 = trunc((x+8)*scale + 0.5) (round-half-up via the cast); clip
    # guards out-of-range inputs from wrapping in the byte split
    _ptsq[:] = _pts_host.reshape(NCORES, NBD, 2, 128, 3).transpose(
        0, 3, 2, 1, 4)
    np.multiply(_ptsq, Q_SCALE, out=_ptsq)
    np.add(_ptsq, 8.0 * Q_SCALE + 0.5, out=_ptsq)
    np.clip(_ptsq, 0.0, 2.0 ** 24 - 1, out=_ptsq)
    v = _ptsq.astype(np.uint32).reshape(NCORES * 128, 96)
    vb8 = v.view(np.uint8).reshape(NCORES * 128, 96, 4)
    _buf_concat[:, 0:96] = vb8[..., 0]
    _buf_concat[:, 96:192] = vb8[..., 1]
    _buf_concat[:, 192:288] = vb8[..., 2]

    fc = [np.asarray(inputs['hand_faces_left']),
          np.asarray(inputs['hand_faces_right'])]
    for s in range(2):
        _faces_concat[:, s, :, :F] = fc[s].T.astype(np.uint8)[None]
    _buf_concat.reshape(NCORES, 128, 312)[:, :, 288:] = \
        _faces_concat.reshape(NCORES, 128, 24)
    return _buf_concat


# --------------------------------------------------------------------------
# device kernel
# --------------------------------------------------------------------------

def _kernel_body(tc, pkd_d, cst_d, extra_d, loss_d):
    import concourse.mybir as mybir
    nc = tc.nc
    fp32 = mybir.dt.float32
    u8 = mybir.dt.uint8
    AF = mybir.ActivationFunctionType
    OP = mybir.AluOpType
    AX = mybir.AxisListType.X

    fp16 = mybir.dt.float16
    with tc.tile_pool(name="const", bufs=1) as cpool:
        lhsT_sb = cpool.tile([5, NBD, PPAD], fp32)
        mrhs_sb = cpool.tile([5, NBD, PPAD], fp32)
        staged = cpool.tile([80, 7, 512], fp32)
        ones = cpool.tile([128, 1], fp32)
        beps = cpool.tile([128, 1], fp32)
        sacc = cpool.tile([128, NBLK], fp32)
        minda = cpool.tile([128, NBLK], fp32)
        nc.vector.memset(ones[:], 1.0)
        nc.vector.memset(beps[:], 1e-12)

        # ---------------- phase 0: derive constants on device ----------
        with tc.tile_pool(name="ph0", bufs=1) as zp:
            ones1 = zp.tile([1, 128], fp32)
            ones1h = zp.tile([1, 128], fp16)
            ones3 = zp.tile([3, 1], fp32)
            twos3 = zp.tile([3, 1], fp32)
            nc.vector.memset(ones1[:], 1.0)
            nc.vector.memset(ones1h[:], 1.0)
            nc.vector.memset(ones3[:], 1.0)
            nc.vector.memset(twos3[:], 2.0)
            pts_sb = zp.tile([128, 2, NBD, 3], fp32)
            pkd_sb = zp.tile([128, 3, 96], u8)
            faces_u8 = zp.tile([1, 2, 3, 512], u8)
            faces_sb = zp.tile([1, 2, 3, 512], fp16)
            cst_sb = zp.tile([128, 8], fp32)
            extra_sb = zp.tile([1, PPAD], fp32)
            nc.sync.dma_start(pkd_sb[:], pkd_d[:, 0:288])
            nc.sync.dma_start(faces_u8[:], pkd_d[:, 288:312])
            nc.sync.dma_start(cst_sb[:], cst_d[:])
            nc.sync.dma_start(extra_sb[:], extra_d[:])
            # decode planar u24: pts = (b0 + b1*256 + b2*65536)*DQ - 8
            ptmp = zp.tile([128, 2, NBD, 3], fp32)
            ptmp2 = zp.tile([128, 2, NBD, 3], fp32)
            nc.vector.scalar_tensor_tensor(ptmp[:], pkd_sb[:, 1:2, :], 256.0,
                                           pkd_sb[:, 0:1, :], OP.mult, OP.add)
            nc.vector.scalar_tensor_tensor(ptmp2[:], pkd_sb[:, 2:3, :],
                                           65536.0, ptmp[:], OP.mult, OP.add)
            nc.scalar.activation(pts_sb[:], ptmp2[:], AF.Copy, bias=-8.0,
                                 scale=DQ_SCALE)
            # faces u8 -> f16 for the broadcast matmuls
            nc.vector.tensor_scalar(faces_sb[:], faces_u8[:], 0.0, None,
                                    OP.add)

            # shared coefficient row (DMA'd into each bd's staged block)
            rc = zp.tile([1, 7, 512], fp32)
            nc.vector.memset(rc[:, 0:3, :], 1.0)
            nc.vector.memset(rc[:, 3:6, :], 2.0)
            nc.vector.memset(rc[:, 6:7, :], 0.0)

            # one-hot face matrices per hand s, corner k, K-chunk kk
            # + identity one-hot (for pts transposition via gather matmul)
            oh = zp.tile([128, 2, 3, 2, 512], fp32)
            idh = zp.tile([128, 2, PPAD], fp32)
            PT = zp.tile([3, PPAD], fp32)
            SQ = zp.tile([3, PPAD], fp32)
            one256 = zp.tile([1, PPAD], fp32)
            nc.vector.memset(one256[:], 1.0)
            with tc.tile_pool(name="ph0bc", bufs=1, space="PSUM") as bp:
                bc = bp.tile([128, 3, 512], fp32)
                bcid = bp.tile([128, PPAD], fp32)
                ptp = bp.tile([3, PPAD], fp32)
                sqp = bp.tile([1, PPAD], fp32)
                for s in range(2):
                    for k in range(3):
                        nc.tensor.matmul(bc[:, k, :], ones1h[:],
                                         faces_sb[:, s, k, :])
                    for k in range(3):
                        for kk in range(2):
                            nc.vector.tensor_scalar(
                                oh[:, s, k, kk, :], bc[:, k, :],
                                cst_sb[:, kk:kk + 1], None, OP.is_equal)
                nc.tensor.matmul(bcid[:], ones1[:], extra_sb[:])
                for kk in range(2):
                    nc.vector.tensor_scalar(idh[:, kk, :], bcid[:],
                                            cst_sb[:, kk:kk + 1], None,
                                            OP.is_equal)
                # lhsT rows from pts: -2*pts^T via identity-gather matmuls,
                # |p|^2 via square + ones3-reduce
                for bd in range(NBD):
                    nc.tensor.matmul(ptp[:], pts_sb[:, 0, bd, :],
                                     idh[:, 0, :], start=True, stop=False)
                    nc.tensor.matmul(ptp[:], pts_sb[:, 1, bd, :],
                                     idh[:, 1, :], start=False, stop=True)
                    nc.scalar.mul(lhsT_sb[0:3, bd, :], ptp[:], -2.0)
                    nc.scalar.activation(PT[:], ptp[:], AF.Copy)
                    nc.vector.tensor_tensor(SQ[:], PT[:], PT[:], OP.mult)
                    nc.tensor.matmul(sqp[:], ones3[:], SQ[:])
                    sq1 = zp.tile([1, PPAD], fp32, name="sq1", tag="sq1",
                                  bufs=2)
                    nc.scalar.activation(sq1[:], sqp[:], AF.Copy)
                    nc.sync.dma_start(lhsT_sb[4:5, bd, :], sq1[:])
                    nc.sync.dma_start(lhsT_sb[3:4, bd, :], one256[:])

            # mrhs: rows0..2 = -0.5*lhsT rows0..2 (= vert xyz),
            # row3 <- lhsT row4 (|v|^2), row4 <- lhsT row3 (ones)
            nc.vector.tensor_scalar(mrhs_sb[0:3], lhsT_sb[0:3], -0.5, None,
                                    OP.mult)
            nc.sync.dma_start(mrhs_sb[3:4], lhsT_sb[4:5])
            nc.sync.dma_start(mrhs_sb[4:5], lhsT_sb[3:4])

            E1 = zp.tile([3, 512], fp32)
            E2 = zp.tile([3, 512], fp32)
            rotc = zp.tile([3, 4, 512], fp32)
            t1 = zp.tile([3, 512], fp32)
            t2 = zp.tile([3, 512], fp32)

            with tc.tile_pool(name="ph0ps", bufs=1, space="PSUM") as pp0:
                crn = [pp0.tile([3, 512], fp32, name=f"crn{t}", tag=t)
                       for t in "abc"]
                rot = pp0.tile([3, 4, 512], fp32)
                c3p = pp0.tile([1, 512], fp32)
                for bd in range(NBD):
                    other = bd ^ 1
                    s = other & 1
                    for k in range(3):
                        nc.tensor.matmul(crn[k][:], pts_sb[:, 0, other, :],
                                         oh[:, s, k, 0, :],
                                         start=True, stop=False)
                        nc.tensor.matmul(crn[k][:], pts_sb[:, 1, other, :],
                                         oh[:, s, k, 1, :],
                                         start=False, stop=True)
                    # double-buffered so bd+1's build overlaps bd's tail
                    asm = zp.tile([3, 7, 512], fp32, name="asm", tag="asm",
                                  bufs=2)
                    PRD = zp.tile([3, 7, 512], fp32, name="PRD", tag="PRD",
                                  bufs=2)
                    C3r = zp.tile([1, 7, 512], fp32, name="C3r", tag="C3r",
                                  bufs=2)
                    Ac, Bc, Cc = (asm[:, 0, :], asm[:, 1, :], asm[:, 2, :])
                    nc.scalar.activation(Ac, crn[0][:], AF.Copy)
                    nc.scalar.activation(Bc, crn[1][:], AF.Copy)
                    nc.scalar.activation(Cc, crn[2][:], AF.Copy)
                    nc.vector.tensor_tensor(E1[:], Bc, Ac, OP.subtract)
                    nc.vector.tensor_tensor(E2[:], Cc, Ac, OP.subtract)
                    # n = E1 x E2 via rotations: rot1/rot2 = P1^T/P2^T
                    nc.tensor.matmul(rot[:, 0, :], cst_sb[0:3, 2:5], E1[:])
                    nc.tensor.matmul(rot[:, 1, :], cst_sb[0:3, 5:8], E2[:])
                    nc.tensor.matmul(rot[:, 2, :], cst_sb[0:3, 5:8], E1[:])
                    nc.tensor.matmul(rot[:, 3, :], cst_sb[0:3, 2:5], E2[:])
                    nc.scalar.activation(rotc[:], rot[:], AF.Copy)
                    nc.vector.tensor_tensor(t1[:], rotc[:, 0, :],
                                            rotc[:, 1, :], OP.mult)
                    nc.vector.tensor_tensor(t2[:], rotc[:, 2, :],
                                            rotc[:, 3, :], OP.mult)
                    nc.vector.tensor_tensor(asm[:, 6, :], t1[:], t2[:],
                                            OP.subtract)
                    # products for the c3 reduces + midpoint sums
                    nc.vector.tensor_tensor(PRD[:, 0, :], Ac, Ac, OP.mult)
                    nc.vector.tensor_tensor(PRD[:, 1, :], Bc, Bc, OP.mult)
                    nc.vector.tensor_tensor(PRD[:, 2, :], Cc, Cc, OP.mult)
                    nc.vector.tensor_tensor(PRD[:, 3, :], Ac, Bc, OP.mult)
                    nc.vector.tensor_tensor(PRD[:, 4, :], Bc, Cc, OP.mult)
                    nc.vector.tensor_tensor(PRD[:, 5, :], Cc, Ac, OP.mult)
                    nc.vector.tensor_tensor(PRD[:, 6, :], Ac, asm[:, 6, :],
                                            OP.mult)
                    nc.vector.tensor_tensor(asm[:, 3, :], Ac, Bc, OP.add)
                    nc.vector.tensor_tensor(asm[:, 4, :], Bc, Cc, OP.add)
                    nc.vector.tensor_tensor(asm[:, 5, :], Cc, Ac, OP.add)
                    for g in range(7):
                        nc.tensor.matmul(c3p[:], ones3[:] if g < 3 else twos3[:],
                                         PRD[:, g, :])
                        nc.scalar.activation(C3r[:, g, :], c3p[:], AF.Copy)
                    # assemble this bd's staged block: xyz rows, c3 row, coeffs
                    nc.sync.dma_start(staged[5 * bd:5 * bd + 3], asm[:])
                    nc.sync.dma_start(staged[5 * bd + 3:5 * bd + 4], C3r[:])
                    nc.sync.dma_start(staged[5 * bd + 4:5 * bd + 5], rc[:])

        # ---------------- phase 1: main compute loop --------------------
        with (
            tc.tile_pool(name="store", bufs=1) as spool,
            tc.tile_pool(name="stage", bufs=2) as stpool,
            tc.tile_pool(name="iface", bufs=2) as ipool,
            tc.tile_pool(name="dve", bufs=1) as vpool,
        ):
            denoms = spool.tile([128, SUPER, 512], fp32)
            tts = spool.tile([128, SUPER, 512], fp32)

            def pass_a(ppool, i, j):
                bd, ch = divmod(i, 2)
                if ch == 0:
                    fstage = stpool.tile([5, 7, 512], fp32, tag="fstage")
                    nc.sync.dma_start(fstage[:], staged[5 * bd:5 * bd + 5])
                    pass_a.stage = fstage
                fstage = pass_a.stage
                lhs = lhsT_sb[:, bd, ch * 128:(ch + 1) * 128]

                wind = ppool.tile([128, 7, 512], fp32, tag="wind")
                md = ppool.tile([128, 256], fp32, tag="md")

                # per-group matmuls pipeline with the ACT/DVE consumers:
                # group 0's results stream downstream while groups 1-6 are
                # still on the PE (a single merged [128, 7x512] matmul
                # simmed 61us SLOWER despite saving 6 fixed overheads)
                for g in range(7):
                    nc.tensor.matmul(wind[:, g, :F], lhs, fstage[:, g, :F])
                nc.tensor.matmul(md[:, :P], lhs, mrhs_sb[:, bd ^ 1, :P])

                # min-distance: free-dim min, clamp at 0 (matmul roundoff)
                mind = vpool.tile([128, 1], fp32, tag="mind")
                nc.vector.tensor_reduce(mind[:], md[:, :P], AX, OP.min)
                nc.vector.tensor_scalar(minda[:, i:i + 1], mind[:], 0.0, None,
                                        OP.max)

                # norms: clamp squared lengths at 0, sqrt (one multi-dim-AP
                # op per stage instead of three)
                rl = ipool.tile([128, 3, 512], fp32, tag="rl")
                nc.scalar.activation(rl[:, :, :F], wind[:, 0:3, :F], AF.Relu)
                sq3 = ipool.tile([128, 3, 512], fp32, tag="sq3")
                nc.scalar.activation(sq3[:, :, :F], rl[:, :, :F], AF.Sqrt)
                la = sq3[:, 0, :]
                lb = sq3[:, 1, :]
                lc = sq3[:, 2, :]
                dets = ipool.tile([128, 512], fp32, tag="dets")
                nc.scalar.mul(dets[:, :F], wind[:, 6, :F], 0.5)

                # denominator chain; wind groups 3..5 hold 2ab/2bc/2ca so
                # fold the x0.5 into the scalar_tensor_tensor ops
                u = vpool.tile([128, 512], fp32, tag="u")
                r4 = vpool.tile([128, 512], fp32, tag="r4")
                s5 = vpool.tile([128, 512], fp32, tag="s5")
                v = vpool.tile([128, 512], fp32, tag="v")
                w = vpool.tile([128, 512], fp32, tag="w")
                t6 = vpool.tile([128, 512], fp32, tag="t6")
                nc.vector.scalar_tensor_tensor(r4[:, :F], wind[:, 4, :F], 0.5,
                                               la[:, :F], OP.mult, OP.mult)
                nc.vector.scalar_tensor_tensor(s5[:, :F], wind[:, 5, :F], 0.5,
                                               lb[:, :F], OP.mult, OP.mult)
                nc.vector.tensor_tensor(u[:, :F], la[:, :F], lb[:, :F], OP.mult)
                nc.vector.scalar_tensor_tensor(v[:, :F], wind[:, 3, :F], 0.5,
                                               u[:, :F], OP.mult, OP.add)

                w_ = w[:, :F]
                nc.vector.tensor_tensor(w_, v[:, :F], lc[:, :F], OP.mult)
                nc.vector.tensor_tensor(t6[:, :F], r4[:, :F], s5[:, :F], OP.add)
                den = denoms[:, j, :F]
                nc.vector.tensor_tensor(den, w_, t6[:, :F], OP.add)

                # half-angle atan2 range reduction: tt = det / (rho + |den|)
                xx = ipool.tile([128, 512], fp32, tag="xx")
                yy = ipool.tile([128, 512], fp32, tag="yy")
                ss = vpool.tile([128, 512], fp32, tag="ss", bufs=2)
                rho = ipool.tile([128, 512], fp32, tag="rho")
                axd = ipool.tile([128, 512], fp32, tag="axd")
                dd = vpool.tile([128, 512], fp32, tag="dd")
                rd = vpool.tile([128, 512], fp32, tag="rd")
                nc.scalar.activation(xx[:, :F], den, AF.Square)
                nc.scalar.activation(yy[:, :F], dets[:, :F], AF.Square)
                nc.vector.scalar_tensor_tensor(ss[:, :F], xx[:, :F], 1e-20,
                                               yy[:, :F], OP.add, OP.add)
                nc.scalar.activation(rho[:, :F], ss[:, :F], AF.Sqrt)
                nc.scalar.activation(axd[:, :F], den, AF.Abs)
                nc.vector.tensor_tensor(dd[:, :F], rho[:, :F], axd[:, :F],
                                        OP.add)
                nc.vector.reciprocal_approx_fast(rd[:, :F], dd[:, :F])
                nc.vector.tensor_tensor(tts[:, j, :F], dets[:, :F], rd[:, :F],
                                        OP.mult)

            def pass_b(i, j):
                den = denoms[:, j, :F]
                tt = tts[:, j, :F]
                sgn = ipool.tile([128, 512], fp32, tag="sgn")
                spi = ipool.tile([128, 512], fp32, tag="spi")
                atn = ipool.tile([128, 512], fp32, tag="atn")
                c0 = vpool.tile([128, 512], fp32, tag="c0")
                c1 = vpool.tile([128, 512], fp32, tag="c1")
                sd = vpool.tile([128, 512], fp32, tag="sd")
                nc.scalar.activation(sgn[:, :F], tt, AF.Sign)
                nc.scalar.mul(spi[:, :F], sgn[:, :F], HALF_PI)
                nc.scalar.activation(atn[:, :F], tt, AF.Arctan)
                # half = atn + [den<0]*(pi/2*sign(det) - 2*atn)
                nc.vector.scalar_tensor_tensor(c0[:, :F], atn[:, :F], -2.0,
                                               spi[:, :F], OP.mult, OP.add)
                nc.vector.scalar_tensor_tensor(c1[:, :F], den, 0.0,
                                               c0[:, :F], OP.is_lt, OP.mult)
                nc.vector.scalar_tensor_tensor(sd[:, :F], atn[:, :F], 0.0,
                                               c1[:, :F], OP.add, OP.add,
                                               accum_out=sacc[:, i:i + 1])

            nc.vector.memset(sacc[:], 0.0)
            nc.vector.memset(minda[:], 1.0)
            with tc.tile_pool(name="psum", bufs=1, space="PSUM") as ppool:
                for sg in range(0 if SKIP_P1 else NBLK // SUPER):
                    for j in range(SUPER):
                        pass_a(ppool, sg * SUPER + j, j)
                    tc.no_sync_barrier()
                    for j in range(SUPER):
                        pass_b(sg * SUPER + j, j)
                    tc.no_sync_barrier()

            # ------------- final: depth * inside, partition-reduce -------
            inside = cpool.tile([128, NBLK], fp32)
            depth = cpool.tile([128, NBLK], fp32)
            contrib = cpool.tile([128, NBLK], fp32)
            nc.vector.tensor_scalar(inside[:], sacc[:], HALF_PI, None,
                                    OP.is_gt)
            nc.scalar.activation(depth[:], minda[:], AF.Sqrt, bias=beps[:])
            nc.vector.tensor_tensor(contrib[:], depth[:], inside[:], OP.mult)

            with tc.tile_pool(name="psum2", bufs=1, space="PSUM") as p2:
                lpsum = p2.tile([NBLK, 1], fp32)
                nc.tensor.matmul(lpsum[:], contrib[:], ones[:])
                loss_sb = cpool.tile([NBLK, 1], fp32)
                nc.scalar.activation(loss_sb[:], lpsum[:], AF.Copy)
                nc.sync.dma_start(loss_d[:], loss_sb[:])


def _build():
    global _compiled
    if _compiled is not None:
        return _compiled
    import concourse.bacc as bacc
    import concourse.mybir as mybir
    import concourse.tile as tile

    nc = bacc.Bacc("TRN2", target_bir_lowering=False, debug=False,
                   num_devices=NCORES)
    fp32 = mybir.dt.float32
    u8 = mybir.dt.uint8
    pkd_d = nc.dram_tensor("pkd", (128, 312), u8, kind="ExternalInput").ap()
    cst_d = nc.dram_tensor("cst", (128, 8), fp32, kind="ExternalInput").ap()
    extra_d = nc.dram_tensor("extra", (1, PPAD), fp32, kind="ExternalInput").ap()
    loss_d = nc.dram_tensor("loss", (NBLK, 1), fp32, kind="ExternalOutput").ap()

    with tile.TileContext(nc) as tc:
        _kernel_body(tc, pkd_d, cst_d, extra_d, loss_d)
    nc.compile()
    _compiled = nc
    return nc


# --------------------------------------------------------------------------
# cached jitted runner + entry point
# --------------------------------------------------------------------------

def _build_runner():
    global _runner
    if _runner is not None:
        return _runner
    import jax
    from jax.sharding import Mesh, PartitionSpec
    from jax.experimental.shard_map import shard_map
    import concourse.mybir as mybir
    from concourse.bass2jax import (_bass_exec_p, partition_id_tensor,
                                    install_neuronx_cc_hook)

    nc = _build()
    install_neuronx_cc_hook()
    pname = nc.partition_id_tensor.name if nc.partition_id_tensor else None
    in_names, out_names, out_avals, zero_outs = [], [], [], []
    for alloc in nc.m.functions[0].allocations:
        if not isinstance(alloc, mybir.MemoryLocationSet):
            continue
        name = alloc.memorylocations[0].name
        if alloc.kind == "ExternalInput":
            if name != pname:
                in_names.append(name)
        elif alloc.kind == "ExternalOutput":
            out_names.append(name)
            shape = tuple(alloc.tensor_shape)
            dtype = mybir.dt.np(alloc.dtype)
            out_avals.append(jax.core.ShapedArray(shape, dtype))
            zero_outs.append(np.zeros(shape, dtype))
    n_params, n_outs = len(in_names), len(out_avals)
    in_names_full = in_names + out_names + ([pname] if pname else [])

    def _body(*args):
        operands = list(args)
        if pname is not None:
            operands.append(partition_id_tensor())
        return tuple(_bass_exec_p.bind(
            *operands, out_avals=tuple(out_avals), in_names=tuple(in_names_full),
            out_names=tuple(out_names), lowering_input_output_aliases=(),
            sim_require_finite=True, sim_require_nnan=True, nc=nc))

    devices = jax.devices()[:NCORES]
    mesh = Mesh(np.asarray(devices), ("core",))
    in_specs = (PartitionSpec("core"),) * (n_params + n_outs)
    out_specs = (PartitionSpec("core"),) * len(out_names)
    # no donation: the kernel writes every output element, so uninit
    # result buffers are fine and the zero operands can stay device-resident
    # forever (their per-call 8-shard upload caused an intermittent ~40ms
    # slow tail)
    sharded = jax.jit(
        shard_map(_body, mesh=mesh, in_specs=in_specs, out_specs=out_specs,
                  check_rep=False),
        keep_unused=True)
    # cst/extra are pure algorithm constants (iota columns, permutation
    # matrices): commit them device-resident once; passing the committed
    # arrays skips their per-call host->device processing (~4ms).
    from jax.sharding import NamedSharding
    shc = NamedSharding(mesh, PartitionSpec("core"))
    resident = {"cst": jax.device_put(_cst_concat, shc),
                "extra": jax.device_put(_extra_concat, shc)}
    rzeros = [jax.device_put(
        np.zeros((NCORES * z.shape[0],) + z.shape[1:], z.dtype), shc)
        for z in zero_outs]
    jax.block_until_ready(list(resident.values()) + rzeros)
    _runner = (sharded, in_names, rzeros, resident, shc)
    return _runner


def kernel(**inputs) -> np.ndarray:
    global last_exec_time_ns
    sharded, in_names, rzeros, resident, shc = _build_runner()
    # single direct 8-way sharded upload: the dev0-then-reshard hop
    # measured 1.3ms SLOWER than direct sharding, and one packed command
    # beats three separate arrays by ~1ms of per-command overhead.
    by_name = {"pkd": _prep(inputs)}
    concat_in = [resident.get(nm, by_name.get(nm)) for nm in in_names]
    out = sharded(*concat_in, *rzeros)
    last_exec_time_ns = None

    # block i = (b_loc*2 + dir)*2 + chunk -> sum each batch's 4 blocks
    o0 = np.asarray(out[0]).reshape(NCORES, NB, 4)
    return np.ascontiguousarray(o0.sum(axis=2).reshape(B).astype(np.float32))



# revision 34
# speedup vs baseline: 1.0397x; 1.0397x over previous
"""Trainium2 Bass kernel for nn_HandIntersectionLoss.

Strategy
--------
Pure data parallel over batch: 64 batches -> 8 cores x 8 local batches.

Wall-clock per call is dominated by the axon tunnel, so the host ships
only the gathered hand points (u24-packed, ~40KB/core) and the device
derives all per-(batch,face) matmul constants itself:

  phase 0 (device):
    - one-hot face matrices from f32 face indices (K=1 broadcast matmul
      + is_equal against shipped iota columns)
    - triangle corners A,B,C per (batch,dir) via 2-chunk accumulated
      gather matmuls:  corners[3,500] = pts[128,3]^T @ onehot[128,500]
    - edges E1=B-A, E2=C-A; normal n = E1 x E2 via permutation-matmul
      rotations (engines cannot read partition offsets != 0)
    - dots |A|^2,.., 2A.B,.., 2A.n via ones/twos-column reduce matmuls
    - constants assembled into a persistent `staged` SBUF tile
      ([65,7,512]: 4 rows per (batch,dir) + shared coefficient row)
      via SBUF->SBUF DMAs (the only legal cross-partition mover)

  phase 1 (device): the proven compute loop. Per 128-point block:
    K=5 matmuls against staged constants produce la^2,lb^2,lc^2,
    2ab,2bc,2ca, 2det for [128 points x 500 faces]; per-element chain
    (denominator + range-reduced atan2) on DVE/ACT:

      atan2(det, den) = 2*atan(det / (rho + |den|))            (den >= 0)
                      = sign(det)*pi - 2*atan(det/(rho+|den|)) (den < 0)
      rho = sqrt(det^2 + den^2 + 1e-20)

    inside(p) <=> sum_f half > pi/2.  Min-distance via the same matmul
    trick against derived vert constants (mrhs) + free-dim min-reduce.
    Scalar-engine table sets force the two-pass structure (sqrt vs
    arctan live in different ACT table sets), staged in super-groups.

The jitted shard_map callable is cached across kernel() calls so repeat
calls skip jax retrace/XLA recompile entirely.

Wire format (wall clock = tunnel RTT ~40ms + payload at ~55-60MB/s, so
payload bytes rule everything): points ship as u24 fixed point over
[-8, 8) (abs step 9.5e-7; loss rel-err 5.2e-3, a plateau set by one
zero-margin inside/outside flip — f16 at 2.6e-2 breaches the 2e-2 gate,
u16 at 1.8e-2 has no margin, u21..u24 are all on the plateau).  Faces
ship as u8 (indices < 251).  Everything packs into ONE u8 array
[1024, 312] uploaded direct 8-way sharded ([128, 312] per core): one
command beats three by ~1ms, and direct sharding beats the old
dev0+reshard hop by ~1.3ms.  Byte planes are PLANAR (b0|b1|b2|faces),
which lets the relay's zstd shrink the low-entropy gaussian b2 plane:
319,488B raw -> ~240,500B on the wire when the compressor keeps up.
Device decodes pts = (b0 + b1*256 + b2*65536) * 16/(2^24-1) - 8 in
three engine ops.

Group semantics (raw, no halving on device):
  g0..2: xyz=A|B|C,       c3=|A|^2..,  w=1
  g3..5: xyz=(A+B)..raw,  c3=2A.B..,   w=2   -> col = 2*(A-p).(B-p)
  g6:    xyz=n raw,       c3=2*A.n,    w=0   -> col = 2*det
pass_a compensates with x0.5 folded into existing scalar_tensor_tensor.
"""
import sys
import numpy as np

sys.path.insert(0, '/opt/trn_rl_repo')

B, V_FULL, V_HAND, V_LOOP, N_FACES = 64, 6890, 250, 20, 500
P = V_HAND + 1          # 251 points/verts per hand (incl. lid)
PPAD = 256
NCORES = 8
NB = B // NCORES        # local batches per core
NBD = NB * 2            # (batch, dir) pairs per core
NBLK = NBD * 2          # blocks per core: x2 point-chunks of 128
SUPER = 8               # blocks per two-pass super-group
F = N_FACES
HALF_PI = float(np.pi / 2)

_compiled = None
SKIP_P1 = False
_runner = None
last_exec_time_ns = None

# u24 fixed point over [-8, 8): x -> round((x+8) * (2^24-1)/16)
Q_SCALE = (2.0 ** 24 - 1) / 16.0
DQ_SCALE = 16.0 / (2.0 ** 24 - 1)


# --------------------------------------------------------------------------
# host prep: index gathers + u24 encode (all heavy constant math on-device)
# --------------------------------------------------------------------------

# preallocated per-call buffers (pad columns written once; concat layouts
# built directly to skip per-core copies).  Pad points sit at 7.5 — far
# outside the unit-scale hand cloud (winding ~0) yet clear of the u24
# top end (8.0 would round to 2^24 in f32 and wrap to -8 in the byte
# split).
#
# Everything dynamic ships in ONE u8 array [1024, 312] (sharded to
# [128, 312] per core) — one command beats three by ~1ms of per-command
# tunnel overhead.  Byte PLANES are kept separate (planar, not
# interleaved triplets): the tunnel zstd-compresses the request, and the
# b2 plane (gaussian high byte, ~5.3 bits entropy) only compresses when
# contiguous.  Unit-stride planes also read faster on the DVE.
#   bytes [:,   0: 96] = b0 (low)   of u24 = round((x+8) * (2^24-1)/16)
#   bytes [:,  96:192] = b1 (mid)
#   bytes [:, 192:288] = b2 (high)
#   bytes [:, 288:312] = faces u8, row-major: buf[p, 288+j] = flat[p*24+j]
_pts_host = np.full((B, 2, PPAD, 3), 7.5, np.float32)
_ptsq = np.empty((NCORES, 128, 2, NBD, 3), np.float32)
_buf_concat = np.empty((NCORES * 128, 312), np.uint8)
_faces_concat = np.zeros((NCORES, 2, 3, 512), np.uint8)
_cst_concat = np.zeros((NCORES * 128, 8), np.float32)
for _c in range(NCORES):
    _cs = _cst_concat[_c * 128:(_c + 1) * 128]
    _cs[:, 0] = np.arange(128, dtype=np.float32)
    _cs[:, 1] = np.arange(128, 256, dtype=np.float32)
    for _m in range(3):
        _cs[(_m + 1) % 3, 2 + _m] = 1.0    # P1 (rot1)
        _cs[(_m + 2) % 3, 5 + _m] = 1.0    # P2 (rot2)
_extra_concat = np.ascontiguousarray(
    np.broadcast_to(np.arange(PPAD, dtype=np.float32), (NCORES, PPAD)))


def _prep(inputs):
    verts = np.asarray(inputs['verts_batch'], dtype=np.float32)
    hi = [np.asarray(inputs['hand_verts_inds_left']),
          np.asarray(inputs['hand_verts_inds_right'])]
    li = [np.asarray(inputs['hand_loop_verts_inds_left']),
          np.asarray(inputs['hand_loop_verts_inds_right'])]

    # pad stays 8.0 from init (pad cols never overwritten)
    for d in range(2):
        _pts_host[:, d, :V_HAND] = verts[:, hi[d]]
        _pts_host[:, d, V_HAND] = verts[:, li[d]].mean(axis=1,
                                                       dtype=np.float32)

    # [core, 128, 2kk, bd, 3] gather layout in one strided copy, then
    # u24 = trunc((x+8)*scale + 0.5) (round-half-up via the cast); clip
    # guards out-of-range inputs from wrapping in the byte split
    _ptsq[:] = _pts_host.reshape(NCORES, NBD, 2, 128, 3).transpose(
        0, 3, 2, 1, 4)
    np.multiply(_ptsq, Q_SCALE, out=_ptsq)
    np.add(_ptsq, 8.0 * Q_SCALE + 0.5, out=_ptsq)
    np.clip(_ptsq, 0.0, 2.0 ** 24 - 1, out=_ptsq)
    v = _ptsq.astype(np.uint32).reshape(NCORES * 128, 96)
    vb8 = v.view(np.uint8).reshape(NCORES * 128, 96, 4)
    _buf_concat[:, 0:96] = vb8[..., 0]
    _buf_concat[:, 96:192] = vb8[..., 1]
    _buf_concat[:, 192:288] = vb8[..., 2]

    fc = [np.asarray(inputs['hand_faces_left']),
          np.asarray(inputs['hand_faces_right'])]
    for s in range(2):
        _faces_concat[:, s, :, :F] = fc[s].T.astype(np.uint8)[None]
    _buf_concat.reshape(NCORES, 128, 312)[:, :, 288:] = \
        _faces_concat.reshape(NCORES, 128, 24)
    return _buf_concat


# --------------------------------------------------------------------------
# device kernel
# --------------------------------------------------------------------------

def _kernel_body(tc, pkd_d, cst_d, extra_d, loss_d):
    import concourse.mybir as mybir
    nc = tc.nc
    fp32 = mybir.dt.float32
    u8 = mybir.dt.uint8
    AF = mybir.ActivationFunctionType
    OP = mybir.AluOpType
    AX = mybir.AxisListType.X

    fp16 = mybir.dt.float16
    with tc.tile_pool(name="const", bufs=1) as cpool:
        lhsT_sb = cpool.tile([5, NBD, PPAD], fp32)
        mrhs_sb = cpool.tile([5, NBD, PPAD], fp32)
        staged = cpool.tile([80, 7, 512], fp32)
        ones = cpool.tile([128, 1], fp32)
        beps = cpool.tile([128, 1], fp32)
        sacc = cpool.tile([128, NBLK], fp32)
        minda = cpool.tile([128, NBLK], fp32)
        nc.vector.memset(ones[:], 1.0)
        nc.vector.memset(beps[:], 1e-12)

        # ---------------- phase 0: derive constants on device ----------
        with tc.tile_pool(name="ph0", bufs=1) as zp:
            ones1 = zp.tile([1, 128], fp32)
            ones1h = zp.tile([1, 128], fp16)
            ones3 = zp.tile([3, 1], fp32)
            twos3 = zp.tile([3, 1], fp32)
            nc.vector.memset(ones1[:], 1.0)
            nc.vector.memset(ones1h[:], 1.0)
            nc.vector.memset(ones3[:], 1.0)
            nc.vector.memset(twos3[:], 2.0)
            pts_sb = zp.tile([128, 2, NBD, 3], fp32)
            pkd_sb = zp.tile([128, 3, 96], u8)
            faces_u8 = zp.tile([1, 2, 3, 512], u8)
            faces_sb = zp.tile([1, 2, 3, 512], fp16)
            cst_sb = zp.tile([128, 8], fp32)
            extra_sb = zp.tile([1, PPAD], fp32)
            nc.sync.dma_start(pkd_sb[:], pkd_d[:, 0:288])
            nc.sync.dma_start(faces_u8[:], pkd_d[:, 288:312])
            nc.sync.dma_start(cst_sb[:], cst_d[:])
            nc.sync.dma_start(extra_sb[:], extra_d[:])
            # decode planar u24: pts = (b0 + b1*256 + b2*65536)*DQ - 8
            ptmp = zp.tile([128, 2, NBD, 3], fp32)
            ptmp2 = zp.tile([128, 2, NBD, 3], fp32)
            nc.vector.scalar_tensor_tensor(ptmp[:], pkd_sb[:, 1:2, :], 256.0,
                                           pkd_sb[:, 0:1, :], OP.mult, OP.add)
            nc.vector.scalar_tensor_tensor(ptmp2[:], pkd_sb[:, 2:3, :],
                                           65536.0, ptmp[:], OP.mult, OP.add)
            nc.scalar.activation(pts_sb[:], ptmp2[:], AF.Copy, bias=-8.0,
                                 scale=DQ_SCALE)
            # faces u8 -> f16 for the broadcast matmuls
            nc.vector.tensor_scalar(faces_sb[:], faces_u8[:], 0.0, None,
                                    OP.add)
            # pts4 = [xyz, |p|^2] per point: extending the corner-gather
            # lhsT to 4 rows makes |A|^2,|B|^2,|C|^2 fall out of the same
            # gather matmuls, killing 3 of the 7 c3 column-sum matmuls
            # per bd (~80us of quarter-rate f32 PE time)
            pts4 = zp.tile([128, 2, NBD, 4], fp32)
            psq = zp.tile([128, 2, NBD, 3], fp32)
            s01 = zp.tile([128, 2, NBD], fp32)
            nc.scalar.activation(pts4[:, :, :, 0:3], pts_sb[:], AF.Copy)
            nc.vector.tensor_tensor(psq[:], pts_sb[:], pts_sb[:], OP.mult)
            nc.vector.tensor_tensor(s01[:], psq[:, :, :, 0], psq[:, :, :, 1],
                                    OP.add)
            nc.vector.tensor_tensor(pts4[:, :, :, 3], s01[:],
                                    psq[:, :, :, 2], OP.add)

            # shared coefficient row (DMA'd into each bd's staged block)
            rc = zp.tile([1, 7, 512], fp32)
            nc.vector.memset(rc[:, 0:3, :], 1.0)
            nc.vector.memset(rc[:, 3:6, :], 2.0)
            nc.vector.memset(rc[:, 6:7, :], 0.0)

            # one-hot face matrices per hand s, corner k, K-chunk kk
            # + identity one-hot (for pts transposition via gather matmul)
            oh = zp.tile([128, 2, 3, 2, 512], fp32)
            idh = zp.tile([128, 2, PPAD], fp32)
            PT = zp.tile([3, PPAD], fp32)
            SQ = zp.tile([3, PPAD], fp32)
            one256 = zp.tile([1, PPAD], fp32)
            nc.vector.memset(one256[:], 1.0)
            with tc.tile_pool(name="ph0bc", bufs=1, space="PSUM") as bp:
                bc = bp.tile([128, 3, 512], fp32)
                bcid = bp.tile([128, PPAD], fp32)
                ptp = bp.tile([3, PPAD], fp32)
                sqp = bp.tile([1, PPAD], fp32)
                for s in range(2):
                    for k in range(3):
                        nc.tensor.matmul(bc[:, k, :], ones1h[:],
                                         faces_sb[:, s, k, :])
                    for k in range(3):
                        for kk in range(2):
                            nc.vector.tensor_scalar(
                                oh[:, s, k, kk, :], bc[:, k, :],
                                cst_sb[:, kk:kk + 1], None, OP.is_equal)
                nc.tensor.matmul(bcid[:], ones1[:], extra_sb[:])
                for kk in range(2):
                    nc.vector.tensor_scalar(idh[:, kk, :], bcid[:],
                                            cst_sb[:, kk:kk + 1], None,
                                            OP.is_equal)
                # lhsT rows from pts: -2*pts^T via identity-gather matmuls,
                # |p|^2 via square + ones3-reduce
                for bd in range(NBD):
                    nc.tensor.matmul(ptp[:], pts_sb[:, 0, bd, :],
                                     idh[:, 0, :], start=True, stop=False)
                    nc.tensor.matmul(ptp[:], pts_sb[:, 1, bd, :],
                                     idh[:, 1, :], start=False, stop=True)
                    nc.scalar.mul(lhsT_sb[0:3, bd, :], ptp[:], -2.0)
                    nc.scalar.activation(PT[:], ptp[:], AF.Copy)
                    nc.vector.tensor_tensor(SQ[:], PT[:], PT[:], OP.mult)
                    nc.tensor.matmul(sqp[:], ones3[:], SQ[:])
                    sq1 = zp.tile([1, PPAD], fp32, name="sq1", tag="sq1",
                                  bufs=2)
                    nc.scalar.activation(sq1[:], sqp[:], AF.Copy)
                    nc.sync.dma_start(lhsT_sb[4:5, bd, :], sq1[:])
                    nc.sync.dma_start(lhsT_sb[3:4, bd, :], one256[:])

            # mrhs: rows0..2 = -0.5*lhsT rows0..2 (= vert xyz),
            # row3 <- lhsT row4 (|v|^2), row4 <- lhsT row3 (ones)
            nc.vector.tensor_scalar(mrhs_sb[0:3], lhsT_sb[0:3], -0.5, None,
                                    OP.mult)
            nc.sync.dma_start(mrhs_sb[3:4], lhsT_sb[4:5])
            nc.sync.dma_start(mrhs_sb[4:5], lhsT_sb[3:4])

            E1 = zp.tile([3, 512], fp32)
            E2 = zp.tile([3, 512], fp32)
            rotc = zp.tile([3, 4, 512], fp32)
            t1 = zp.tile([3, 512], fp32)
            t2 = zp.tile([3, 512], fp32)

            with tc.tile_pool(name="ph0ps", bufs=1, space="PSUM") as pp0:
                crn = [pp0.tile([4, 512], fp32, name=f"crn{t}", tag=t)
                       for t in "abc"]
                rot = pp0.tile([3, 4, 512], fp32)
                c3p = pp0.tile([1, 512], fp32)
                for bd in range(NBD):
                    other = bd ^ 1
                    s = other & 1
                    for k in range(3):
                        nc.tensor.matmul(crn[k][:], pts4[:, 0, other, :],
                                         oh[:, s, k, 0, :],
                                         start=True, stop=False)
                        nc.tensor.matmul(crn[k][:], pts4[:, 1, other, :],
                                         oh[:, s, k, 1, :],
                                         start=False, stop=True)
                    # double-buffered so bd+1's build overlaps bd's tail.
                    # asm4[:, k, :] = [xyz, |corner|^2] of corner k (the
                    # full 4-row PSUM drain keeps the ACT read at
                    # partition base 0; row 3 then rides a plain SBUF DMA
                    # into the staged c3 row).  asm holds groups 3..6.
                    asm4 = zp.tile([4, 3, 512], fp32, name="asm4",
                                   tag="asm4", bufs=2)
                    asm = zp.tile([3, 4, 512], fp32, name="asm", tag="asm",
                                  bufs=2)
                    PRD = zp.tile([3, 4, 512], fp32, name="PRD", tag="PRD",
                                  bufs=2)
                    C3r = zp.tile([1, 4, 512], fp32, name="C3r", tag="C3r",
                                  bufs=2)
                    nc.scalar.activation(asm4[:, 0, :], crn[0][:], AF.Copy)
                    nc.scalar.activation(asm4[:, 1, :], crn[1][:], AF.Copy)
                    nc.scalar.activation(asm4[:, 2, :], crn[2][:], AF.Copy)
                    Ac = asm4[0:3, 0, :]
                    Bc = asm4[0:3, 1, :]
                    Cc = asm4[0:3, 2, :]
                    nc.vector.tensor_tensor(E1[:], Bc, Ac, OP.subtract)
                    nc.vector.tensor_tensor(E2[:], Cc, Ac, OP.subtract)
                    # n = E1 x E2 via rotations: rot1/rot2 = P1^T/P2^T
                    nc.tensor.matmul(rot[:, 0, :], cst_sb[0:3, 2:5], E1[:])
                    nc.tensor.matmul(rot[:, 1, :], cst_sb[0:3, 5:8], E2[:])
                    nc.tensor.matmul(rot[:, 2, :], cst_sb[0:3, 5:8], E1[:])
                    nc.tensor.matmul(rot[:, 3, :], cst_sb[0:3, 2:5], E2[:])
                    nc.scalar.activation(rotc[:], rot[:], AF.Copy)
                    nc.vector.tensor_tensor(t1[:], rotc[:, 0, :],
                                            rotc[:, 1, :], OP.mult)
                    nc.vector.tensor_tensor(t2[:], rotc[:, 2, :],
                                            rotc[:, 3, :], OP.mult)
                    nc.vector.tensor_tensor(asm[:, 3, :], t1[:], t2[:],
                                            OP.subtract)
                    # products for the remaining c3 reduces + midpoint sums
                    # (g0..2 = |A|^2.. rode the corner gathers)
                    nc.vector.tensor_tensor(PRD[:, 0, :], Ac, Bc, OP.mult)
                    nc.vector.tensor_tensor(PRD[:, 1, :], Bc, Cc, OP.mult)
                    nc.vector.tensor_tensor(PRD[:, 2, :], Cc, Ac, OP.mult)
                    nc.vector.tensor_tensor(PRD[:, 3, :], Ac, asm[:, 3, :],
                                            OP.mult)
                    nc.vector.tensor_tensor(asm[:, 0, :], Ac, Bc, OP.add)
                    nc.vector.tensor_tensor(asm[:, 1, :], Bc, Cc, OP.add)
                    nc.vector.tensor_tensor(asm[:, 2, :], Cc, Ac, OP.add)
                    for g in range(4):
                        nc.tensor.matmul(c3p[:], twos3[:], PRD[:, g, :])
                        nc.scalar.activation(C3r[:, g, :], c3p[:], AF.Copy)
                    # assemble this bd's staged block: xyz rows (corners
                    # from asm4, groups 3..6 from asm), c3 row (|corner|^2
                    # from asm4 row 3, rest from C3r), coeffs
                    nc.sync.dma_start(staged[5 * bd:5 * bd + 3, 0:3],
                                      asm4[0:3, :, :])
                    nc.sync.dma_start(staged[5 * bd:5 * bd + 3, 3:7],
                                      asm[:])
                    nc.sync.dma_start(staged[5 * bd + 3:5 * bd + 4, 0:3],
                                      asm4[3:4, :, :])
                    nc.sync.dma_start(staged[5 * bd + 3:5 * bd + 4, 3:7],
                                      C3r[:])
                    nc.sync.dma_start(staged[5 * bd + 4:5 * bd + 5], rc[:])

        # ---------------- phase 1: main compute loop --------------------
        with (
            tc.tile_pool(name="store", bufs=1) as spool,
            tc.tile_pool(name="stage", bufs=2) as stpool,
            tc.tile_pool(name="iface", bufs=2) as ipool,
            tc.tile_pool(name="dve", bufs=1) as vpool,
        ):
            denoms = spool.tile([128, SUPER, 512], fp32)
            tts = spool.tile([128, SUPER, 512], fp32)

            def pass_a(ppool, i, j):
                bd, ch = divmod(i, 2)
                if ch == 0:
                    fstage = stpool.tile([5, 7, 512], fp32, tag="fstage")
                    nc.sync.dma_start(fstage[:], staged[5 * bd:5 * bd + 5])
                    pass_a.stage = fstage
                fstage = pass_a.stage
                lhs = lhsT_sb[:, bd, ch * 128:(ch + 1) * 128]

                wind = ppool.tile([128, 7, 512], fp32, tag="wind")
                md = ppool.tile([128, 256], fp32, tag="md")

                # per-group matmuls pipeline with the ACT/DVE consumers:
                # group 0's results stream downstream while groups 1-6 are
                # still on the PE (a single merged [128, 7x512] matmul
                # simmed 61us SLOWER despite saving 6 fixed overheads)
                for g in range(7):
                    nc.tensor.matmul(wind[:, g, :F], lhs, fstage[:, g, :F])
                nc.tensor.matmul(md[:, :P], lhs, mrhs_sb[:, bd ^ 1, :P])

                # min-distance: free-dim min, clamp at 0 (matmul roundoff)
                mind = vpool.tile([128, 1], fp32, tag="mind")
                nc.vector.tensor_reduce(mind[:], md[:, :P], AX, OP.min)
                nc.vector.tensor_scalar(minda[:, i:i + 1], mind[:], 0.0, None,
                                        OP.max)

                # norms: clamp squared lengths at 0, sqrt (one multi-dim-AP
                # op per stage instead of three)
                rl = ipool.tile([128, 3, 512], fp32, tag="rl")
                nc.scalar.activation(rl[:, :, :F], wind[:, 0:3, :F], AF.Relu)
                sq3 = ipool.tile([128, 3, 512], fp32, tag="sq3")
                nc.scalar.activation(sq3[:, :, :F], rl[:, :, :F], AF.Sqrt)
                la = sq3[:, 0, :]
                lb = sq3[:, 1, :]
                lc = sq3[:, 2, :]
                dets = ipool.tile([128, 512], fp32, tag="dets")
                nc.scalar.mul(dets[:, :F], wind[:, 6, :F], 0.5)

                # denominator chain; wind groups 3..5 hold 2ab/2bc/2ca so
                # fold the x0.5 into the scalar_tensor_tensor ops
                u = vpool.tile([128, 512], fp32, tag="u")
                r4 = vpool.tile([128, 512], fp32, tag="r4")
                s5 = vpool.tile([128, 512], fp32, tag="s5")
                v = vpool.tile([128, 512], fp32, tag="v")
                w = vpool.tile([128, 512], fp32, tag="w")
                t6 = vpool.tile([128, 512], fp32, tag="t6")
                nc.vector.scalar_tensor_tensor(r4[:, :F], wind[:, 4, :F], 0.5,
                                               la[:, :F], OP.mult, OP.mult)
                nc.vector.scalar_tensor_tensor(s5[:, :F], wind[:, 5, :F], 0.5,
                                               lb[:, :F], OP.mult, OP.mult)
                nc.vector.tensor_tensor(u[:, :F], la[:, :F], lb[:, :F], OP.mult)
                nc.vector.scalar_tensor_tensor(v[:, :F], wind[:, 3, :F], 0.5,
                                               u[:, :F], OP.mult, OP.add)

                w_ = w[:, :F]
                nc.vector.tensor_tensor(w_, v[:, :F], lc[:, :F], OP.mult)
                nc.vector.tensor_tensor(t6[:, :F], r4[:, :F], s5[:, :F], OP.add)
                den = denoms[:, j, :F]
                nc.vector.tensor_tensor(den, w_, t6[:, :F], OP.add)

                # half-angle atan2 range reduction: tt = det / (rho + |den|)
                xx = ipool.tile([128, 512], fp32, tag="xx")
                yy = ipool.tile([128, 512], fp32, tag="yy")
                ss = vpool.tile([128, 512], fp32, tag="ss", bufs=2)
                rho = ipool.tile([128, 512], fp32, tag="rho")
                axd = ipool.tile([128, 512], fp32, tag="axd")
                dd = vpool.tile([128, 512], fp32, tag="dd")
                rd = vpool.tile([128, 512], fp32, tag="rd")
                nc.scalar.activation(xx[:, :F], den, AF.Square)
                nc.scalar.activation(yy[:, :F], dets[:, :F], AF.Square)
                nc.vector.scalar_tensor_tensor(ss[:, :F], xx[:, :F], 1e-20,
                                               yy[:, :F], OP.add, OP.add)
                nc.scalar.activation(rho[:, :F], ss[:, :F], AF.Sqrt)
                nc.scalar.activation(axd[:, :F], den, AF.Abs)
                nc.vector.tensor_tensor(dd[:, :F], rho[:, :F], axd[:, :F],
                                        OP.add)
                nc.vector.reciprocal_approx_fast(rd[:, :F], dd[:, :F])
                nc.vector.tensor_tensor(tts[:, j, :F], dets[:, :F], rd[:, :F],
                                        OP.mult)

            def pass_b(i, j):
                den = denoms[:, j, :F]
                tt = tts[:, j, :F]
                sgn = ipool.tile([128, 512], fp32, tag="sgn")
                spi = ipool.tile([128, 512], fp32, tag="spi")
                atn = ipool.tile([128, 512], fp32, tag="atn")
                c0 = vpool.tile([128, 512], fp32, tag="c0")
                c1 = vpool.tile([128, 512], fp32, tag="c1")
                sd = vpool.tile([128, 512], fp32, tag="sd")
                nc.scalar.activation(sgn[:, :F], tt, AF.Sign)
                nc.scalar.mul(spi[:, :F], sgn[:, :F], HALF_PI)
                nc.scalar.activation(atn[:, :F], tt, AF.Arctan)
                # half = atn + [den<0]*(pi/2*sign(det) - 2*atn)
                nc.vector.scalar_tensor_tensor(c0[:, :F], atn[:, :F], -2.0,
                                               spi[:, :F], OP.mult, OP.add)
                nc.vector.scalar_tensor_tensor(c1[:, :F], den, 0.0,
                                               c0[:, :F], OP.is_lt, OP.mult)
                nc.vector.scalar_tensor_tensor(sd[:, :F], atn[:, :F], 0.0,
                                               c1[:, :F], OP.add, OP.add,
                                               accum_out=sacc[:, i:i + 1])

            nc.vector.memset(sacc[:], 0.0)
            nc.vector.memset(minda[:], 1.0)
            with tc.tile_pool(name="psum", bufs=1, space="PSUM") as ppool:
                for sg in range(0 if SKIP_P1 else NBLK // SUPER):
                    for j in range(SUPER):
                        pass_a(ppool, sg * SUPER + j, j)
                    tc.no_sync_barrier()
                    for j in range(SUPER):
                        pass_b(sg * SUPER + j, j)
                    tc.no_sync_barrier()

            # ------------- final: depth * inside, partition-reduce -------
            inside = cpool.tile([128, NBLK], fp32)
            depth = cpool.tile([128, NBLK], fp32)
            contrib = cpool.tile([128, NBLK], fp32)
            nc.vector.tensor_scalar(inside[:], sacc[:], HALF_PI, None,
                                    OP.is_gt)
            nc.scalar.activation(depth[:], minda[:], AF.Sqrt, bias=beps[:])
            nc.vector.tensor_tensor(contrib[:], depth[:], inside[:], OP.mult)

            with tc.tile_pool(name="psum2", bufs=1, space="PSUM") as p2:
                lpsum = p2.tile([NBLK, 1], fp32)
                nc.tensor.matmul(lpsum[:], contrib[:], ones[:])
                loss_sb = cpool.tile([NBLK, 1], fp32)
                nc.scalar.activation(loss_sb[:], lpsum[:], AF.Copy)
                nc.sync.dma_start(loss_d[:], loss_sb[:])


def _build():
    global _compiled
    if _compiled is not None:
        return _compiled
    import concourse.bacc as bacc
    import concourse.mybir as mybir
    import concourse.tile as tile

    nc = bacc.Bacc("TRN2", target_bir_lowering=False, debug=False,
                   num_devices=NCORES)
    fp32 = mybir.dt.float32
    u8 = mybir.dt.uint8
    pkd_d = nc.dram_tensor("pkd", (128, 312), u8, kind="ExternalInput").ap()
    cst_d = nc.dram_tensor("cst", (128, 8), fp32, kind="ExternalInput").ap()
    extra_d = nc.dram_tensor("extra", (1, PPAD), fp32, kind="ExternalInput").ap()
    loss_d = nc.dram_tensor("loss", (NBLK, 1), fp32, kind="ExternalOutput").ap()

    with tile.TileContext(nc) as tc:
        _kernel_body(tc, pkd_d, cst_d, extra_d, loss_d)
    nc.compile()
    _compiled = nc
    return nc


# --------------------------------------------------------------------------
# cached jitted runner + entry point
# --------------------------------------------------------------------------

def _build_runner():
    global _runner
    if _runner is not None:
        return _runner
    import jax
    from jax.sharding import Mesh, PartitionSpec
    from jax.experimental.shard_map import shard_map
    import concourse.mybir as mybir
    from concourse.bass2jax import (_bass_exec_p, partition_id_tensor,
                                    install_neuronx_cc_hook)

    nc = _build()
    install_neuronx_cc_hook()
    pname = nc.partition_id_tensor.name if nc.partition_id_tensor else None
    in_names, out_names, out_avals, zero_outs = [], [], [], []
    for alloc in nc.m.functions[0].allocations:
        if not isinstance(alloc, mybir.MemoryLocationSet):
            continue
        name = alloc.memorylocations[0].name
        if alloc.kind == "ExternalInput":
            if name != pname:
                in_names.append(name)
        elif alloc.kind == "ExternalOutput":
            out_names.append(name)
            shape = tuple(alloc.tensor_shape)
            dtype = mybir.dt.np(alloc.dtype)
            out_avals.append(jax.core.ShapedArray(shape, dtype))
            zero_outs.append(np.zeros(shape, dtype))
    n_params, n_outs = len(in_names), len(out_avals)
    in_names_full = in_names + out_names + ([pname] if pname else [])

    def _body(*args):
        operands = list(args)
        if pname is not None:
            operands.append(partition_id_tensor())
        return tuple(_bass_exec_p.bind(
            *operands, out_avals=tuple(out_avals), in_names=tuple(in_names_full),
            out_names=tuple(out_names), lowering_input_output_aliases=(),
            sim_require_finite=True, sim_require_nnan=True, nc=nc))

    devices = jax.devices()[:NCORES]
    mesh = Mesh(np.asarray(devices), ("core",))
    in_specs = (PartitionSpec("core"),) * (n_params + n_outs)
    out_specs = (PartitionSpec("core"),) * len(out_names)
    # no donation: the kernel writes every output element, so uninit
    # result buffers are fine and the zero operands can stay device-resident
    # forever (their per-call 8-shard upload caused an intermittent ~40ms
    # slow tail)
    sharded = jax.jit(
        shard_map(_body, mesh=mesh, in_specs=in_specs, out_specs=out_specs,
                  check_rep=False),
        keep_unused=True)
    # cst/extra are pure algorithm constants (iota columns, permutation
    # matrices): commit them device-resident once; passing the committed
    # arrays skips their per-call host->device processing (~4ms).
    from jax.sharding import NamedSharding
    shc = NamedSharding(mesh, PartitionSpec("core"))
    resident = {"cst": jax.device_put(_cst_concat, shc),
                "extra": jax.device_put(_extra_concat, shc)}
    rzeros = [jax.device_put(
        np.zeros((NCORES * z.shape[0],) + z.shape[1:], z.dtype), shc)
        for z in zero_outs]
    jax.block_until_ready(list(resident.values()) + rzeros)
    _runner = (sharded, in_names, rzeros, resident, shc)
    return _runner


def kernel(**inputs) -> np.ndarray:
    global last_exec_time_ns
    sharded, in_names, rzeros, resident, shc = _build_runner()
    # single direct 8-way sharded upload: the dev0-then-reshard hop
    # measured 1.3ms SLOWER than direct sharding, and one packed command
    # beats three separate arrays by ~1ms of per-command overhead.
    by_name = {"pkd": _prep(inputs)}
    concat_in = [resident.get(nm, by_name.get(nm)) for nm in in_names]
    out = sharded(*concat_in, *rzeros)
    last_exec_time_ns = None

    # block i = (b_loc*2 + dir)*2 + chunk -> sum each batch's 4 blocks
    o0 = np.asarray(out[0]).reshape(NCORES, NB, 4)
    return np.ascontiguousarray(o0.sum(axis=2).reshape(B).astype(np.float32))



# revision 43
# speedup vs baseline: 1.0753x; 1.0342x over previous
"""Trainium2 Bass kernel for nn_HandIntersectionLoss.

Strategy
--------
Pure data parallel over batch: 64 batches -> 8 cores x 8 local batches.

Wall-clock per call is dominated by the axon tunnel, so the host ships
only the gathered hand points (u24-packed, ~40KB/core) and the device
derives all per-(batch,face) matmul constants itself:

  phase 0 (device):
    - one-hot face matrices from f32 face indices (K=1 broadcast matmul
      + is_equal against shipped iota columns)
    - triangle corners A,B,C per (batch,dir) via 2-chunk accumulated
      gather matmuls:  corners[3,500] = pts[128,3]^T @ onehot[128,500]
    - edges E1=B-A, E2=C-A; normal n = E1 x E2 via permutation-matmul
      rotations (engines cannot read partition offsets != 0)
    - dots |A|^2,.., 2A.B,.., 2A.n via ones/twos-column reduce matmuls
    - constants assembled into a persistent `staged` SBUF tile
      ([65,7,512]: 4 rows per (batch,dir) + shared coefficient row)
      via SBUF->SBUF DMAs (the only legal cross-partition mover)

  phase 1 (device): the proven compute loop. Per 128-point block:
    K=5 matmuls against staged constants produce la^2,lb^2,lc^2,
    2ab,2bc,2ca, 2det for [128 points x 500 faces]; per-element chain
    (denominator + range-reduced atan2) on DVE/ACT:

      atan2(det, den) = 2*atan(det / (rho + |den|))            (den >= 0)
                      = sign(det)*pi - 2*atan(det/(rho+|den|)) (den < 0)
      rho = sqrt(det^2 + den^2 + 1e-20)

    inside(p) <=> sum_f half > pi/2.  Min-distance via the same matmul
    trick against derived vert constants (mrhs) + free-dim min-reduce.
    Scalar-engine table sets force the two-pass structure (sqrt vs
    arctan live in different ACT table sets), staged in super-groups.

The jitted shard_map callable is cached across kernel() calls so repeat
calls skip jax retrace/XLA recompile entirely.

Wire format (wall clock = tunnel RTT ~40ms + payload at ~55-60MB/s, so
payload bytes rule everything): points ship as u24 fixed point over
[-8, 8) (abs step 9.5e-7; loss rel-err 5.2e-3, a plateau set by one
zero-margin inside/outside flip — f16 at 2.6e-2 breaches the 2e-2 gate,
u16 at 1.8e-2 has no margin, u21..u24 are all on the plateau).  Faces
ship as u8 (indices < 251).  Everything packs into ONE u8 array
[1024, 312] uploaded direct 8-way sharded ([128, 312] per core): one
command beats three by ~1ms, and direct sharding beats the old
dev0+reshard hop by ~1.3ms.  Byte planes are PLANAR (b0|b1|b2|faces),
which lets the relay's zstd shrink the low-entropy gaussian b2 plane:
319,488B raw -> ~240,500B on the wire when the compressor keeps up.
Device decodes pts = (b0 + b1*256 + b2*65536) * 16/(2^24-1) - 8 in
three engine ops.

Group semantics (raw, no halving on device):
  g0..2: xyz=A|B|C,       c3=|A|^2..,  w=1
  g3..5: xyz=(A+B)..raw,  c3=2A.B..,   w=2   -> col = 2*(A-p).(B-p)
  g6:    xyz=n raw,       c3=2*A.n,    w=0   -> col = 2*det
pass_a compensates with x0.5 folded into existing scalar_tensor_tensor.
"""
import sys
import numpy as np

sys.path.insert(0, '/opt/trn_rl_repo')

B, V_FULL, V_HAND, V_LOOP, N_FACES = 64, 6890, 250, 20, 500
P = V_HAND + 1          # 251 points/verts per hand (incl. lid)
PPAD = 256
NCORES = 8
NB = B // NCORES        # local batches per core
NBD = NB * 2            # (batch, dir) pairs per core
NBLK = NBD * 2          # blocks per core: x2 point-chunks of 128
SUPER = 8               # blocks per two-pass super-group
F = N_FACES
HALF_PI = float(np.pi / 2)

_compiled = None
SKIP_P1 = False
_runner = None
last_exec_time_ns = None

# u24 fixed point over [-8, 8): x -> round((x+8) * (2^24-1)/16)
Q_SCALE = (2.0 ** 24 - 1) / 16.0
DQ_SCALE = 16.0 / (2.0 ** 24 - 1)


# --------------------------------------------------------------------------
# host prep: index gathers + u24 encode (all heavy constant math on-device)
# --------------------------------------------------------------------------

# preallocated per-call buffers (pad columns written once; concat layouts
# built directly to skip per-core copies).  Pad points sit at 7.5 — far
# outside the unit-scale hand cloud (winding ~0) yet clear of the u24
# top end (8.0 would round to 2^24 in f32 and wrap to -8 in the byte
# split).
#
# Everything dynamic ships in ONE u8 array [1024, 312] (sharded to
# [128, 312] per core) — one command beats three by ~1ms of per-command
# tunnel overhead.  Byte PLANES are kept separate (planar, not
# interleaved triplets): the tunnel zstd-compresses the request, and the
# b2 plane (gaussian high byte, ~5.3 bits entropy) only compresses when
# contiguous.  Unit-stride planes also read faster on the DVE.
#   bytes [:,   0: 96] = b0 (low)   of u24 = round((x+8) * (2^24-1)/16)
#   bytes [:,  96:192] = b1 (mid)
#   bytes [:, 192:288] = b2 (high)
#   bytes [:, 288:312] = faces u8, row-major: buf[p, 288+j] = flat[p*24+j]
_pts_host = np.full((B, 2, PPAD, 3), 7.5, np.float32)
_ptsq = np.empty((NCORES, 128, 2, NBD, 3), np.float32)
_buf_concat = np.empty((NCORES * 128, 312), np.uint8)
_faces_concat = np.zeros((NCORES, 2, 3, 512), np.uint8)
_cst_concat = np.zeros((NCORES * 128, 8), np.float32)
for _c in range(NCORES):
    _cs = _cst_concat[_c * 128:(_c + 1) * 128]
    _cs[:, 0] = np.arange(128, dtype=np.float32)
    _cs[:, 1] = np.arange(128, 256, dtype=np.float32)
    for _m in range(3):
        _cs[(_m + 1) % 3, 2 + _m] = 1.0    # P1 (rot1)
        _cs[(_m + 2) % 3, 5 + _m] = 1.0    # P2 (rot2)
_extra_concat = np.ascontiguousarray(
    np.broadcast_to(np.arange(PPAD, dtype=np.float32), (NCORES, PPAD)))


def _prep(inputs):
    verts = np.asarray(inputs['verts_batch'], dtype=np.float32)
    hi = [np.asarray(inputs['hand_verts_inds_left']),
          np.asarray(inputs['hand_verts_inds_right'])]
    li = [np.asarray(inputs['hand_loop_verts_inds_left']),
          np.asarray(inputs['hand_loop_verts_inds_right'])]

    # pad stays 8.0 from init (pad cols never overwritten)
    for d in range(2):
        _pts_host[:, d, :V_HAND] = verts[:, hi[d]]
        _pts_host[:, d, V_HAND] = verts[:, li[d]].mean(axis=1,
                                                       dtype=np.float32)

    # [core, 128, 2kk, bd, 3] gather layout in one strided copy, then
    # u24 = trunc((x+8)*scale + 0.5) (round-half-up via the cast); clip
    # guards out-of-range inputs from wrapping in the byte split
    _ptsq[:] = _pts_host.reshape(NCORES, NBD, 2, 128, 3).transpose(
        0, 3, 2, 1, 4)
    np.multiply(_ptsq, Q_SCALE, out=_ptsq)
    np.add(_ptsq, 8.0 * Q_SCALE + 0.5, out=_ptsq)
    np.clip(_ptsq, 0.0, 2.0 ** 24 - 1, out=_ptsq)
    v = _ptsq.astype(np.uint32).reshape(NCORES * 128, 96)
    vb8 = v.view(np.uint8).reshape(NCORES * 128, 96, 4)
    _buf_concat[:, 0:96] = vb8[..., 0]
    _buf_concat[:, 96:192] = vb8[..., 1]
    _buf_concat[:, 192:288] = vb8[..., 2]

    fc = [np.asarray(inputs['hand_faces_left']),
          np.asarray(inputs['hand_faces_right'])]
    for s in range(2):
        _faces_concat[:, s, :, :F] = fc[s].T.astype(np.uint8)[None]
    _buf_concat.reshape(NCORES, 128, 312)[:, :, 288:] = \
        _faces_concat.reshape(NCORES, 128, 24)
    return _buf_concat


# --------------------------------------------------------------------------
# device kernel
# --------------------------------------------------------------------------

def _kernel_body(tc, pkd_d, cst_d, extra_d, loss_d):
    import concourse.mybir as mybir
    nc = tc.nc
    fp32 = mybir.dt.float32
    u8 = mybir.dt.uint8
    AF = mybir.ActivationFunctionType
    OP = mybir.AluOpType
    AX = mybir.AxisListType.X

    fp16 = mybir.dt.float16
    with tc.tile_pool(name="const", bufs=1) as cpool:
        lhsT_sb = cpool.tile([5, NBD, PPAD], fp32)
        mrhs_sb = cpool.tile([5, NBD, PPAD], fp32)
        staged = cpool.tile([80, 7, 512], fp32)
        ones = cpool.tile([128, 1], fp32)
        beps = cpool.tile([128, 1], fp32)
        sacc = cpool.tile([128, NBLK], fp32)
        minda = cpool.tile([128, NBLK], fp32)
        nc.vector.memset(ones[:], 1.0)
        nc.vector.memset(beps[:], 1e-12)

        # ---------------- phase 0: derive constants on device ----------
        with tc.tile_pool(name="ph0", bufs=1) as zp:
            ones1 = zp.tile([1, 128], fp32)
            ones1h = zp.tile([1, 128], fp16)
            ones3 = zp.tile([3, 1], fp32)
            twos3 = zp.tile([3, 1], fp32)
            nc.vector.memset(ones1[:], 1.0)
            nc.vector.memset(ones1h[:], 1.0)
            nc.vector.memset(ones3[:], 1.0)
            nc.vector.memset(twos3[:], 2.0)
            pts_sb = zp.tile([128, 2, NBD, 3], fp32)
            pkd_sb = zp.tile([128, 3, 96], u8)
            faces_u8 = zp.tile([1, 2, 3, 512], u8)
            faces_sb = zp.tile([1, 2, 3, 512], fp16)
            cst_sb = zp.tile([128, 8], fp32)
            extra_sb = zp.tile([1, PPAD], fp32)
            nc.sync.dma_start(pkd_sb[:], pkd_d[:, 0:288])
            nc.sync.dma_start(faces_u8[:], pkd_d[:, 288:312])
            nc.sync.dma_start(cst_sb[:], cst_d[:])
            nc.sync.dma_start(extra_sb[:], extra_d[:])
            # decode planar u24: pts = (b0 + b1*256 + b2*65536)*DQ - 8
            ptmp = zp.tile([128, 2, NBD, 3], fp32)
            ptmp2 = zp.tile([128, 2, NBD, 3], fp32)
            nc.vector.scalar_tensor_tensor(ptmp[:], pkd_sb[:, 1:2, :], 256.0,
                                           pkd_sb[:, 0:1, :], OP.mult, OP.add)
            nc.vector.scalar_tensor_tensor(ptmp2[:], pkd_sb[:, 2:3, :],
                                           65536.0, ptmp[:], OP.mult, OP.add)
            nc.scalar.activation(pts_sb[:], ptmp2[:], AF.Copy, bias=-8.0,
                                 scale=DQ_SCALE)
            # faces u8 -> f16 for the broadcast matmuls
            nc.vector.tensor_scalar(faces_sb[:], faces_u8[:], 0.0, None,
                                    OP.add)
            # pts4 = [xyz, |p|^2] per point: extending the corner-gather
            # lhsT to 4 rows makes |A|^2,|B|^2,|C|^2 fall out of the same
            # gather matmuls, killing 3 of the 7 c3 column-sum matmuls
            # per bd (~80us of quarter-rate f32 PE time)
            pts4 = zp.tile([128, 2, NBD, 4], fp32)
            psq = zp.tile([128, 2, NBD, 3], fp32)
            s01 = zp.tile([128, 2, NBD], fp32)
            nc.scalar.activation(pts4[:, :, :, 0:3], pts_sb[:], AF.Copy)
            nc.vector.tensor_tensor(psq[:], pts_sb[:], pts_sb[:], OP.mult)
            nc.vector.tensor_tensor(s01[:], psq[:, :, :, 0], psq[:, :, :, 1],
                                    OP.add)
            nc.vector.tensor_tensor(pts4[:, :, :, 3], s01[:],
                                    psq[:, :, :, 2], OP.add)

            # shared coefficient row (DMA'd into each bd's staged block)
            rc = zp.tile([1, 7, 512], fp32)
            nc.vector.memset(rc[:, 0:3, :], 1.0)
            nc.vector.memset(rc[:, 3:6, :], 2.0)
            nc.vector.memset(rc[:, 6:7, :], 0.0)

            # one-hot face matrices per hand s, corner k, K-chunk kk
            # + identity one-hot (for pts transposition via gather matmul)
            oh = zp.tile([128, 2, 3, 2, 512], fp32)
            idh = zp.tile([128, 2, PPAD], fp32)
            PT = zp.tile([3, PPAD], fp32)
            SQ = zp.tile([3, PPAD], fp32)
            one256 = zp.tile([1, PPAD], fp32)
            nc.vector.memset(one256[:], 1.0)
            with tc.tile_pool(name="ph0bc", bufs=1, space="PSUM") as bp:
                bc = bp.tile([128, 3, 512], fp32)
                bcid = bp.tile([128, PPAD], fp32)
                ptp = bp.tile([3, PPAD], fp32)
                sqp = bp.tile([1, PPAD], fp32)
                for s in range(2):
                    for k in range(3):
                        nc.tensor.matmul(bc[:, k, :], ones1h[:],
                                         faces_sb[:, s, k, :])
                    for k in range(3):
                        for kk in range(2):
                            nc.vector.tensor_scalar(
                                oh[:, s, k, kk, :], bc[:, k, :],
                                cst_sb[:, kk:kk + 1], None, OP.is_equal)
                nc.tensor.matmul(bcid[:], ones1[:], extra_sb[:])
                for kk in range(2):
                    nc.vector.tensor_scalar(idh[:, kk, :], bcid[:],
                                            cst_sb[:, kk:kk + 1], None,
                                            OP.is_equal)
                # lhsT rows from pts: -2*pts^T via identity-gather matmuls,
                # |p|^2 via square + ones3-reduce
                for bd in range(NBD):
                    nc.tensor.matmul(ptp[:], pts_sb[:, 0, bd, :],
                                     idh[:, 0, :], start=True, stop=False)
                    nc.tensor.matmul(ptp[:], pts_sb[:, 1, bd, :],
                                     idh[:, 1, :], start=False, stop=True)
                    nc.scalar.mul(lhsT_sb[0:3, bd, :], ptp[:], -2.0)
                    nc.scalar.activation(PT[:], ptp[:], AF.Copy)
                    nc.vector.tensor_tensor(SQ[:], PT[:], PT[:], OP.mult)
                    nc.tensor.matmul(sqp[:], ones3[:], SQ[:])
                    sq1 = zp.tile([1, PPAD], fp32, name="sq1", tag="sq1",
                                  bufs=2)
                    nc.scalar.activation(sq1[:], sqp[:], AF.Copy)
                    nc.sync.dma_start(lhsT_sb[4:5, bd, :], sq1[:])
                    nc.sync.dma_start(lhsT_sb[3:4, bd, :], one256[:])

            # mrhs: rows0..2 = -0.5*lhsT rows0..2 (= vert xyz),
            # row3 <- lhsT row4 (|v|^2), row4 <- lhsT row3 (ones)
            nc.vector.tensor_scalar(mrhs_sb[0:3], lhsT_sb[0:3], -0.5, None,
                                    OP.mult)
            nc.sync.dma_start(mrhs_sb[3:4], lhsT_sb[4:5])
            nc.sync.dma_start(mrhs_sb[4:5], lhsT_sb[3:4])

            E1 = zp.tile([3, 512], fp32)
            E2 = zp.tile([3, 512], fp32)
            rotc = zp.tile([3, 4, 512], fp32)
            t1 = zp.tile([3, 512], fp32)
            t2 = zp.tile([3, 512], fp32)

            with tc.tile_pool(name="ph0ps", bufs=1, space="PSUM") as pp0:
                crn = [pp0.tile([4, 512], fp32, name=f"crn{t}", tag=t)
                       for t in "abc"]
                rot = pp0.tile([3, 4, 512], fp32)
                c3p = pp0.tile([1, 512], fp32)
                for bd in range(NBD):
                    other = bd ^ 1
                    s = other & 1
                    for k in range(3):
                        nc.tensor.matmul(crn[k][:], pts4[:, 0, other, :],
                                         oh[:, s, k, 0, :],
                                         start=True, stop=False)
                        nc.tensor.matmul(crn[k][:], pts4[:, 1, other, :],
                                         oh[:, s, k, 1, :],
                                         start=False, stop=True)
                    # double-buffered so bd+1's build overlaps bd's tail.
                    # asm4[:, k, :] = [xyz, |corner|^2] of corner k (the
                    # full 4-row PSUM drain keeps the ACT read at
                    # partition base 0; row 3 then rides a plain SBUF DMA
                    # into the staged c3 row).  asm holds groups 3..6.
                    asm4 = zp.tile([4, 3, 512], fp32, name="asm4",
                                   tag="asm4", bufs=2)
                    asm = zp.tile([3, 4, 512], fp32, name="asm", tag="asm",
                                  bufs=2)
                    PRD = zp.tile([3, 4, 512], fp32, name="PRD", tag="PRD",
                                  bufs=2)
                    C3r = zp.tile([1, 4, 512], fp32, name="C3r", tag="C3r",
                                  bufs=2)
                    nc.scalar.activation(asm4[:, 0, :], crn[0][:], AF.Copy)
                    nc.scalar.activation(asm4[:, 1, :], crn[1][:], AF.Copy)
                    nc.scalar.activation(asm4[:, 2, :], crn[2][:], AF.Copy)
                    Ac = asm4[0:3, 0, :]
                    Bc = asm4[0:3, 1, :]
                    Cc = asm4[0:3, 2, :]
                    nc.vector.tensor_tensor(E1[:], Bc, Ac, OP.subtract)
                    nc.vector.tensor_tensor(E2[:], Cc, Ac, OP.subtract)
                    # n = E1 x E2 via rotations: rot1/rot2 = P1^T/P2^T
                    nc.tensor.matmul(rot[:, 0, :], cst_sb[0:3, 2:5], E1[:])
                    nc.tensor.matmul(rot[:, 1, :], cst_sb[0:3, 5:8], E2[:])
                    nc.tensor.matmul(rot[:, 2, :], cst_sb[0:3, 5:8], E1[:])
                    nc.tensor.matmul(rot[:, 3, :], cst_sb[0:3, 2:5], E2[:])
                    nc.scalar.activation(rotc[:], rot[:], AF.Copy)
                    nc.vector.tensor_tensor(t1[:], rotc[:, 0, :],
                                            rotc[:, 1, :], OP.mult)
                    nc.vector.tensor_tensor(t2[:], rotc[:, 2, :],
                                            rotc[:, 3, :], OP.mult)
                    nc.vector.tensor_tensor(asm[:, 3, :], t1[:], t2[:],
                                            OP.subtract)
                    # products for the remaining c3 reduces + midpoint sums
                    # (g0..2 = |A|^2.. rode the corner gathers)
                    nc.vector.tensor_tensor(PRD[:, 0, :], Ac, Bc, OP.mult)
                    nc.vector.tensor_tensor(PRD[:, 1, :], Bc, Cc, OP.mult)
                    nc.vector.tensor_tensor(PRD[:, 2, :], Cc, Ac, OP.mult)
                    nc.vector.tensor_tensor(PRD[:, 3, :], Ac, asm[:, 3, :],
                                            OP.mult)
                    nc.vector.tensor_tensor(asm[:, 0, :], Ac, Bc, OP.add)
                    nc.vector.tensor_tensor(asm[:, 1, :], Bc, Cc, OP.add)
                    nc.vector.tensor_tensor(asm[:, 2, :], Cc, Ac, OP.add)
                    for g in range(4):
                        nc.tensor.matmul(c3p[:], twos3[:], PRD[:, g, :])
                        nc.scalar.activation(C3r[:, g, :], c3p[:], AF.Copy)
                    # assemble this bd's staged block: xyz rows (corners
                    # from asm4, groups 3..6 from asm), c3 row (|corner|^2
                    # from asm4 row 3, rest from C3r), coeffs
                    nc.sync.dma_start(staged[5 * bd:5 * bd + 3, 0:3],
                                      asm4[0:3, :, :])
                    nc.sync.dma_start(staged[5 * bd:5 * bd + 3, 3:7],
                                      asm[:])
                    nc.sync.dma_start(staged[5 * bd + 3:5 * bd + 4, 0:3],
                                      asm4[3:4, :, :])
                    nc.sync.dma_start(staged[5 * bd + 3:5 * bd + 4, 3:7],
                                      C3r[:])
                    nc.sync.dma_start(staged[5 * bd + 4:5 * bd + 5], rc[:])

        # ---------------- phase 1: main compute loop --------------------
        with (
            tc.tile_pool(name="store", bufs=1) as spool,
            tc.tile_pool(name="stage", bufs=2) as stpool,
            tc.tile_pool(name="iface", bufs=2) as ipool,
            tc.tile_pool(name="dve", bufs=1) as vpool,
        ):
            denoms = spool.tile([128, SUPER, 512], fp32)
            tts = spool.tile([128, SUPER, 512], fp32)

            def pass_a(ppool, i, j):
                bd, ch = divmod(i, 2)
                if ch == 0:
                    fstage = stpool.tile([5, 7, 512], fp32, tag="fstage")
                    nc.sync.dma_start(fstage[:], staged[5 * bd:5 * bd + 5])
                    pass_a.stage = fstage
                fstage = pass_a.stage
                lhs = lhsT_sb[:, bd, ch * 128:(ch + 1) * 128]

                wind = ppool.tile([128, 7, 512], fp32, tag="wind")
                md = ppool.tile([128, 256], fp32, tag="md")

                # per-group matmuls pipeline with the ACT/DVE consumers:
                # group 0's results stream downstream while groups 1-6 are
                # still on the PE (a single merged [128, 7x512] matmul
                # simmed 61us SLOWER despite saving 6 fixed overheads)
                for g in range(7):
                    nc.tensor.matmul(wind[:, g, :F], lhs, fstage[:, g, :F])
                nc.tensor.matmul(md[:, :P], lhs, mrhs_sb[:, bd ^ 1, :P])

                # early PSUM drain: groups 3..5 are otherwise read only
                # mid-DVE-chain, holding the bufs=1 wind tile (14KB of the
                # 16KB PSUM) and stalling the next block's matmuls.  With
                # this copy + rl + dets all early ACT ops, PSUM frees
                # ~2.6us after the matmuls instead of ~6us later.
                windc3 = ipool.tile([128, 3, 512], fp32, tag="windc3")
                nc.scalar.activation(windc3[:, :, :F], wind[:, 3:6, :F],
                                     AF.Copy)

                # min-distance: free-dim min, clamp at 0 (matmul roundoff)
                mind = vpool.tile([128, 1], fp32, tag="mind")
                nc.vector.tensor_reduce(mind[:], md[:, :P], AX, OP.min)
                nc.vector.tensor_scalar(minda[:, i:i + 1], mind[:], 0.0, None,
                                        OP.max)

                # norms: clamp squared lengths at 0, sqrt (one multi-dim-AP
                # op per stage instead of three).  dets reads wind BEFORE
                # sq3 (which doesn't) so the PSUM hold ends early.
                rl = ipool.tile([128, 3, 512], fp32, tag="rl")
                nc.scalar.activation(rl[:, :, :F], wind[:, 0:3, :F], AF.Relu)
                dets = ipool.tile([128, 512], fp32, tag="dets")
                nc.scalar.mul(dets[:, :F], wind[:, 6, :F], 0.5)
                sq3 = ipool.tile([128, 3, 512], fp32, tag="sq3")
                nc.scalar.activation(sq3[:, :, :F], rl[:, :, :F], AF.Sqrt)
                la = sq3[:, 0, :]
                lb = sq3[:, 1, :]
                lc = sq3[:, 2, :]

                # denominator chain; wind groups 3..5 hold 2ab/2bc/2ca so
                # fold the x0.5 into the scalar_tensor_tensor ops
                u = vpool.tile([128, 512], fp32, tag="u")
                r4 = vpool.tile([128, 512], fp32, tag="r4")
                s5 = vpool.tile([128, 512], fp32, tag="s5")
                v = vpool.tile([128, 512], fp32, tag="v")
                w = vpool.tile([128, 512], fp32, tag="w")
                t6 = vpool.tile([128, 512], fp32, tag="t6")
                nc.vector.scalar_tensor_tensor(r4[:, :F], windc3[:, 1, :F],
                                               0.5, la[:, :F], OP.mult,
                                               OP.mult)
                nc.vector.scalar_tensor_tensor(s5[:, :F], windc3[:, 2, :F],
                                               0.5, lb[:, :F], OP.mult,
                                               OP.mult)
                nc.vector.tensor_tensor(u[:, :F], la[:, :F], lb[:, :F], OP.mult)
                nc.vector.scalar_tensor_tensor(v[:, :F], windc3[:, 0, :F],
                                               0.5, u[:, :F], OP.mult,
                                               OP.add)

                w_ = w[:, :F]
                nc.vector.tensor_tensor(w_, v[:, :F], lc[:, :F], OP.mult)
                nc.vector.tensor_tensor(t6[:, :F], r4[:, :F], s5[:, :F], OP.add)
                den = denoms[:, j, :F]
                nc.vector.tensor_tensor(den, w_, t6[:, :F], OP.add)

                # half-angle atan2 range reduction: tt = det / (rho + |den|)
                xx = ipool.tile([128, 512], fp32, tag="xx")
                yy = ipool.tile([128, 512], fp32, tag="yy")
                ss = vpool.tile([128, 512], fp32, tag="ss", bufs=2)
                rho = ipool.tile([128, 512], fp32, tag="rho")
                axd = ipool.tile([128, 512], fp32, tag="axd")
                dd = vpool.tile([128, 512], fp32, tag="dd")
                rd = vpool.tile([128, 512], fp32, tag="rd")
                nc.scalar.activation(xx[:, :F], den, AF.Square)
                nc.scalar.activation(yy[:, :F], dets[:, :F], AF.Square)
                nc.vector.scalar_tensor_tensor(ss[:, :F], xx[:, :F], 1e-20,
                                               yy[:, :F], OP.add, OP.add)
                nc.scalar.activation(rho[:, :F], ss[:, :F], AF.Sqrt)
                nc.scalar.activation(axd[:, :F], den, AF.Abs)
                nc.vector.tensor_tensor(dd[:, :F], rho[:, :F], axd[:, :F],
                                        OP.add)
                nc.vector.reciprocal_approx_fast(rd[:, :F], dd[:, :F])
                nc.vector.tensor_tensor(tts[:, j, :F], dets[:, :F], rd[:, :F],
                                        OP.mult)

            def pass_b(i, j):
                den = denoms[:, j, :F]
                tt = tts[:, j, :F]
                sgn = ipool.tile([128, 512], fp32, tag="sgn")
                spi = ipool.tile([128, 512], fp32, tag="spi")
                atn = ipool.tile([128, 512], fp32, tag="atn")
                c0 = vpool.tile([128, 512], fp32, tag="c0")
                c1 = vpool.tile([128, 512], fp32, tag="c1")
                sd = vpool.tile([128, 512], fp32, tag="sd")
                nc.scalar.activation(sgn[:, :F], tt, AF.Sign)
                nc.scalar.mul(spi[:, :F], sgn[:, :F], HALF_PI)
                nc.scalar.activation(atn[:, :F], tt, AF.Arctan)
                # half = atn + [den<0]*(pi/2*sign(det) - 2*atn)
                nc.vector.scalar_tensor_tensor(c0[:, :F], atn[:, :F], -2.0,
                                               spi[:, :F], OP.mult, OP.add)
                nc.vector.scalar_tensor_tensor(c1[:, :F], den, 0.0,
                                               c0[:, :F], OP.is_lt, OP.mult)
                nc.vector.scalar_tensor_tensor(sd[:, :F], atn[:, :F], 0.0,
                                               c1[:, :F], OP.add, OP.add,
                                               accum_out=sacc[:, i:i + 1])

            nc.vector.memset(sacc[:], 0.0)
            nc.vector.memset(minda[:], 1.0)
            with tc.tile_pool(name="psum", bufs=1, space="PSUM") as ppool:
                for sg in range(0 if SKIP_P1 else NBLK // SUPER):
                    for j in range(SUPER):
                        pass_a(ppool, sg * SUPER + j, j)
                    tc.no_sync_barrier()
                    for j in range(SUPER):
                        pass_b(sg * SUPER + j, j)
                    tc.no_sync_barrier()

            # ------------- final: depth * inside, partition-reduce -------
            inside = cpool.tile([128, NBLK], fp32)
            depth = cpool.tile([128, NBLK], fp32)
            contrib = cpool.tile([128, NBLK], fp32)
            nc.vector.tensor_scalar(inside[:], sacc[:], HALF_PI, None,
                                    OP.is_gt)
            nc.scalar.activation(depth[:], minda[:], AF.Sqrt, bias=beps[:])
            nc.vector.tensor_tensor(contrib[:], depth[:], inside[:], OP.mult)

            with tc.tile_pool(name="psum2", bufs=1, space="PSUM") as p2:
                lpsum = p2.tile([NBLK, 1], fp32)
                nc.tensor.matmul(lpsum[:], contrib[:], ones[:])
                loss_sb = cpool.tile([NBLK, 1], fp32)
                nc.scalar.activation(loss_sb[:], lpsum[:], AF.Copy)
                nc.sync.dma_start(loss_d[:], loss_sb[:])


def _build():
    global _compiled
    if _compiled is not None:
        return _compiled
    import concourse.bacc as bacc
    import concourse.mybir as mybir
    import concourse.tile as tile

    nc = bacc.Bacc("TRN2", target_bir_lowering=False, debug=False,
                   num_devices=NCORES)
    fp32 = mybir.dt.float32
    u8 = mybir.dt.uint8
    pkd_d = nc.dram_tensor("pkd", (128, 312), u8, kind="ExternalInput").ap()
    cst_d = nc.dram_tensor("cst", (128, 8), fp32, kind="ExternalInput").ap()
    extra_d = nc.dram_tensor("extra", (1, PPAD), fp32, kind="ExternalInput").ap()
    loss_d = nc.dram_tensor("loss", (NBLK, 1), fp32, kind="ExternalOutput").ap()

    with tile.TileContext(nc) as tc:
        _kernel_body(tc, pkd_d, cst_d, extra_d, loss_d)
    nc.compile()
    _compiled = nc
    return nc


# --------------------------------------------------------------------------
# cached jitted runner + entry point
# --------------------------------------------------------------------------

def _build_runner():
    global _runner
    if _runner is not None:
        return _runner
    import jax
    from jax.sharding import Mesh, PartitionSpec
    from jax.experimental.shard_map import shard_map
    import concourse.mybir as mybir
    from concourse.bass2jax import (_bass_exec_p, partition_id_tensor,
                                    install_neuronx_cc_hook)

    nc = _build()
    install_neuronx_cc_hook()
    pname = nc.partition_id_tensor.name if nc.partition_id_tensor else None
    in_names, out_names, out_avals, zero_outs = [], [], [], []
    for alloc in nc.m.functions[0].allocations:
        if not isinstance(alloc, mybir.MemoryLocationSet):
            continue
        name = alloc.memorylocations[0].name
        if alloc.kind == "ExternalInput":
            if name != pname:
                in_names.append(name)
        elif alloc.kind == "ExternalOutput":
            out_names.append(name)
            shape = tuple(alloc.tensor_shape)
            dtype = mybir.dt.np(alloc.dtype)
            out_avals.append(jax.core.ShapedArray(shape, dtype))
            zero_outs.append(np.zeros(shape, dtype))
    n_params, n_outs = len(in_names), len(out_avals)
    in_names_full = in_names + out_names + ([pname] if pname else [])

    def _body(*args):
        operands = list(args)
        if pname is not None:
            operands.append(partition_id_tensor())
        return tuple(_bass_exec_p.bind(
            *operands, out_avals=tuple(out_avals), in_names=tuple(in_names_full),
            out_names=tuple(out_names), lowering_input_output_aliases=(),
            sim_require_finite=True, sim_require_nnan=True, nc=nc))

    devices = jax.devices()[:NCORES]
    mesh = Mesh(np.asarray(devices), ("core",))
    in_specs = (PartitionSpec("core"),) * (n_params + n_outs)
    out_specs = (PartitionSpec("core"),) * len(out_names)
    # no donation: the kernel writes every output element, so uninit
    # result buffers are fine and the zero operands can stay device-resident
    # forever (their per-call 8-shard upload caused an intermittent ~40ms
    # slow tail)
    sharded = jax.jit(
        shard_map(_body, mesh=mesh, in_specs=in_specs, out_specs=out_specs,
                  check_rep=False),
        keep_unused=True)
    # cst/extra are pure algorithm constants (iota columns, permutation
    # matrices): commit them device-resident once; passing the committed
    # arrays skips their per-call host->device processing (~4ms).
    from jax.sharding import NamedSharding
    shc = NamedSharding(mesh, PartitionSpec("core"))
    resident = {"cst": jax.device_put(_cst_concat, shc),
                "extra": jax.device_put(_extra_concat, shc)}
    rzeros = [jax.device_put(
        np.zeros((NCORES * z.shape[0],) + z.shape[1:], z.dtype), shc)
        for z in zero_outs]
    jax.block_until_ready(list(resident.values()) + rzeros)
    _runner = (sharded, in_names, rzeros, resident, shc)
    return _runner


def kernel(**inputs) -> np.ndarray:
    global last_exec_time_ns
    sharded, in_names, rzeros, resident, shc = _build_runner()
    # single direct 8-way sharded upload: the dev0-then-reshard hop
    # measured 1.3ms SLOWER than direct sharding, and one packed command
    # beats three separate arrays by ~1ms of per-command overhead.
    by_name = {"pkd": _prep(inputs)}
    concat_in = [resident.get(nm, by_name.get(nm)) for nm in in_names]
    out = sharded(*concat_in, *rzeros)
    last_exec_time_ns = None

    # block i = (b_loc*2 + dir)*2 + chunk -> sum each batch's 4 blocks
    o0 = np.asarray(out[0]).reshape(NCORES, NB, 4)
    return np.ascontiguousarray(o0.sum(axis=2).reshape(B).astype(np.float32))



# revision 44
# speedup vs baseline: 1.0959x; 1.0192x over previous
"""Trainium2 Bass kernel for nn_HandIntersectionLoss.

Strategy
--------
Pure data parallel over batch: 64 batches -> 8 cores x 8 local batches.

Wall-clock per call is dominated by the axon tunnel, so the host ships
only the gathered hand points (u24-packed, ~40KB/core) and the device
derives all per-(batch,face) matmul constants itself:

  phase 0 (device):
    - one-hot face matrices from f32 face indices (K=1 broadcast matmul
      + is_equal against shipped iota columns)
    - triangle corners A,B,C per (batch,dir) via 2-chunk accumulated
      gather matmuls:  corners[3,500] = pts[128,3]^T @ onehot[128,500]
    - edges E1=B-A, E2=C-A; normal n = E1 x E2 via permutation-matmul
      rotations (engines cannot read partition offsets != 0)
    - dots |A|^2,.., 2A.B,.., 2A.n via ones/twos-column reduce matmuls
    - constants assembled into a persistent `staged` SBUF tile
      ([65,7,512]: 4 rows per (batch,dir) + shared coefficient row)
      via SBUF->SBUF DMAs (the only legal cross-partition mover)

  phase 1 (device): the proven compute loop. Per 128-point block:
    K=5 matmuls against staged constants produce la^2,lb^2,lc^2,
    2ab,2bc,2ca, 2det for [128 points x 500 faces]; per-element chain
    (denominator + range-reduced atan2) on DVE/ACT:

      atan2(det, den) = 2*atan(det / (rho + |den|))            (den >= 0)
                      = sign(det)*pi - 2*atan(det/(rho+|den|)) (den < 0)
      rho = sqrt(det^2 + den^2 + 1e-20)

    inside(p) <=> sum_f half > pi/2.  Min-distance via the same matmul
    trick against derived vert constants (mrhs) + free-dim min-reduce.
    Scalar-engine table sets force the two-pass structure (sqrt vs
    arctan live in different ACT table sets), staged in super-groups.

The jitted shard_map callable is cached across kernel() calls so repeat
calls skip jax retrace/XLA recompile entirely.

Wire format (wall clock = tunnel RTT ~40ms + payload at ~55-60MB/s, so
payload bytes rule everything): points ship as u24 fixed point over
[-8, 8) (abs step 9.5e-7; loss rel-err 5.2e-3, a plateau set by one
zero-margin inside/outside flip — f16 at 2.6e-2 breaches the 2e-2 gate,
u16 at 1.8e-2 has no margin, u21..u24 are all on the plateau).  Faces
ship as u8 (indices < 251).  Everything packs into ONE u8 array
[1024, 312] uploaded direct 8-way sharded ([128, 312] per core): one
command beats three by ~1ms, and direct sharding beats the old
dev0+reshard hop by ~1.3ms.  Byte planes are PLANAR (b0|b1|b2|faces),
which lets the relay's zstd shrink the low-entropy gaussian b2 plane:
319,488B raw -> ~240,500B on the wire when the compressor keeps up.
Device decodes pts = (b0 + b1*256 + b2*65536) * 16/(2^24-1) - 8 in
three engine ops.

Group semantics (raw, no halving on device):
  g0..2: xyz=A|B|C,       c3=|A|^2..,  w=1
  g3..5: xyz=(A+B)..raw,  c3=2A.B..,   w=2   -> col = 2*(A-p).(B-p)
  g6:    xyz=n raw,       c3=2*A.n,    w=0   -> col = 2*det
pass_a compensates with x0.5 folded into existing scalar_tensor_tensor.
"""
import sys
import numpy as np

sys.path.insert(0, '/opt/trn_rl_repo')

B, V_FULL, V_HAND, V_LOOP, N_FACES = 64, 6890, 250, 20, 500
P = V_HAND + 1          # 251 points/verts per hand (incl. lid)
PPAD = 256
NCORES = 8
NB = B // NCORES        # local batches per core
NBD = NB * 2            # (batch, dir) pairs per core
NBLK = NBD * 2          # blocks per core: x2 point-chunks of 128
SUPER = 8               # blocks per two-pass super-group
F = N_FACES
HALF_PI = float(np.pi / 2)

_compiled = None
SKIP_P1 = False
_runner = None
last_exec_time_ns = None

# u24 fixed point over [-8, 8): x -> round((x+8) * (2^24-1)/16)
Q_SCALE = (2.0 ** 24 - 1) / 16.0
DQ_SCALE = 16.0 / (2.0 ** 24 - 1)


# --------------------------------------------------------------------------
# host prep: index gathers + u24 encode (all heavy constant math on-device)
# --------------------------------------------------------------------------

# preallocated per-call buffers (pad columns written once; concat layouts
# built directly to skip per-core copies).  Pad points sit at 7.5 — far
# outside the unit-scale hand cloud (winding ~0) yet clear of the u24
# top end (8.0 would round to 2^24 in f32 and wrap to -8 in the byte
# split).
#
# Everything dynamic ships in ONE u8 array [1024, 312] (sharded to
# [128, 312] per core) — one command beats three by ~1ms of per-command
# tunnel overhead.  Byte PLANES are kept separate (planar, not
# interleaved triplets): the tunnel zstd-compresses the request, and the
# b2 plane (gaussian high byte, ~5.3 bits entropy) only compresses when
# contiguous.  Unit-stride planes also read faster on the DVE.
#   bytes [:,   0: 96] = b0 (low)   of u24 = round((x+8) * (2^24-1)/16)
#   bytes [:,  96:192] = b1 (mid)
#   bytes [:, 192:288] = b2 (high)
#   bytes [:, 288:312] = faces u8, row-major: buf[p, 288+j] = flat[p*24+j]
_pts_host = np.full((B, 2, PPAD, 3), 7.5, np.float32)
_ptsq = np.empty((NCORES, 128, 2, NBD, 3), np.float32)
_buf_concat = np.empty((NCORES * 128, 312), np.uint8)
_faces_concat = np.zeros((NCORES, 2, 3, 512), np.uint8)
_cst_concat = np.zeros((NCORES * 128, 8), np.float32)
for _c in range(NCORES):
    _cs = _cst_concat[_c * 128:(_c + 1) * 128]
    _cs[:, 0] = np.arange(128, dtype=np.float32)
    _cs[:, 1] = np.arange(128, 256, dtype=np.float32)
    for _m in range(3):
        _cs[(_m + 1) % 3, 2 + _m] = 1.0    # P1 (rot1)
        _cs[(_m + 2) % 3, 5 + _m] = 1.0    # P2 (rot2)
_extra_concat = np.ascontiguousarray(
    np.broadcast_to(np.arange(PPAD, dtype=np.float32), (NCORES, PPAD)))


def _prep(inputs):
    verts = np.asarray(inputs['verts_batch'], dtype=np.float32)
    hi = [np.asarray(inputs['hand_verts_inds_left']),
          np.asarray(inputs['hand_verts_inds_right'])]
    li = [np.asarray(inputs['hand_loop_verts_inds_left']),
          np.asarray(inputs['hand_loop_verts_inds_right'])]

    # pad stays 8.0 from init (pad cols never overwritten)
    for d in range(2):
        _pts_host[:, d, :V_HAND] = verts[:, hi[d]]
        _pts_host[:, d, V_HAND] = verts[:, li[d]].mean(axis=1,
                                                       dtype=np.float32)

    # [core, 128, 2kk, bd, 3] gather layout in one strided copy, then
    # u24 = trunc((x+8)*scale + 0.5) (round-half-up via the cast); clip
    # guards out-of-range inputs from wrapping in the byte split
    _ptsq[:] = _pts_host.reshape(NCORES, NBD, 2, 128, 3).transpose(
        0, 3, 2, 1, 4)
    np.multiply(_ptsq, Q_SCALE, out=_ptsq)
    np.add(_ptsq, 8.0 * Q_SCALE + 0.5, out=_ptsq)
    np.clip(_ptsq, 0.0, 2.0 ** 24 - 1, out=_ptsq)
    v = _ptsq.astype(np.uint32).reshape(NCORES * 128, 96)
    vb8 = v.view(np.uint8).reshape(NCORES * 128, 96, 4)
    _buf_concat[:, 0:96] = vb8[..., 0]
    _buf_concat[:, 96:192] = vb8[..., 1]
    _buf_concat[:, 192:288] = vb8[..., 2]

    fc = [np.asarray(inputs['hand_faces_left']),
          np.asarray(inputs['hand_faces_right'])]
    for s in range(2):
        _faces_concat[:, s, :, :F] = fc[s].T.astype(np.uint8)[None]
    _buf_concat.reshape(NCORES, 128, 312)[:, :, 288:] = \
        _faces_concat.reshape(NCORES, 128, 24)
    return _buf_concat


# --------------------------------------------------------------------------
# device kernel
# --------------------------------------------------------------------------

def _kernel_body(tc, pkd_d, cst_d, extra_d, loss_d):
    import concourse.mybir as mybir
    nc = tc.nc
    fp32 = mybir.dt.float32
    u8 = mybir.dt.uint8
    AF = mybir.ActivationFunctionType
    OP = mybir.AluOpType
    AX = mybir.AxisListType.X

    fp16 = mybir.dt.float16
    with tc.tile_pool(name="const", bufs=1) as cpool:
        lhsT_sb = cpool.tile([5, NBD, PPAD], fp32)
        mrhs_sb = cpool.tile([5, NBD, PPAD], fp32)
        staged = cpool.tile([80, 7, 512], fp32)
        ones = cpool.tile([128, 1], fp32)
        beps = cpool.tile([128, 1], fp32)
        sacc = cpool.tile([128, NBLK], fp32)
        minda = cpool.tile([128, NBLK], fp32)
        nc.vector.memset(ones[:], 1.0)
        nc.vector.memset(beps[:], 1e-12)

        # ---------------- phase 0: derive constants on device ----------
        with tc.tile_pool(name="ph0", bufs=1) as zp:
            ones1 = zp.tile([1, 128], fp32)
            ones1h = zp.tile([1, 128], fp16)
            ones3 = zp.tile([3, 1], fp32)
            twos3 = zp.tile([3, 1], fp32)
            nc.vector.memset(ones1[:], 1.0)
            nc.vector.memset(ones1h[:], 1.0)
            nc.vector.memset(ones3[:], 1.0)
            nc.vector.memset(twos3[:], 2.0)
            pts_sb = zp.tile([128, 2, NBD, 3], fp32)
            pkd_sb = zp.tile([128, 3, 96], u8)
            faces_u8 = zp.tile([1, 2, 3, 512], u8)
            faces_sb = zp.tile([1, 2, 3, 512], fp16)
            cst_sb = zp.tile([128, 8], fp32)
            extra_sb = zp.tile([1, PPAD], fp32)
            nc.sync.dma_start(pkd_sb[:], pkd_d[:, 0:288])
            nc.sync.dma_start(faces_u8[:], pkd_d[:, 288:312])
            nc.sync.dma_start(cst_sb[:], cst_d[:])
            nc.sync.dma_start(extra_sb[:], extra_d[:])
            # decode planar u24: pts = (b0 + b1*256 + b2*65536)*DQ - 8
            ptmp = zp.tile([128, 2, NBD, 3], fp32)
            ptmp2 = zp.tile([128, 2, NBD, 3], fp32)
            nc.vector.scalar_tensor_tensor(ptmp[:], pkd_sb[:, 1:2, :], 256.0,
                                           pkd_sb[:, 0:1, :], OP.mult, OP.add)
            nc.vector.scalar_tensor_tensor(ptmp2[:], pkd_sb[:, 2:3, :],
                                           65536.0, ptmp[:], OP.mult, OP.add)
            nc.scalar.activation(pts_sb[:], ptmp2[:], AF.Copy, bias=-8.0,
                                 scale=DQ_SCALE)
            # faces u8 -> f16 for the broadcast matmuls
            nc.vector.tensor_scalar(faces_sb[:], faces_u8[:], 0.0, None,
                                    OP.add)
            # pts4 = [xyz, |p|^2] per point: extending the corner-gather
            # lhsT to 4 rows makes |A|^2,|B|^2,|C|^2 fall out of the same
            # gather matmuls, killing 3 of the 7 c3 column-sum matmuls
            # per bd (~80us of quarter-rate f32 PE time)
            pts4 = zp.tile([128, 2, NBD, 4], fp32)
            psq = zp.tile([128, 2, NBD, 3], fp32)
            s01 = zp.tile([128, 2, NBD], fp32)
            nc.scalar.activation(pts4[:, :, :, 0:3], pts_sb[:], AF.Copy)
            nc.vector.tensor_tensor(psq[:], pts_sb[:], pts_sb[:], OP.mult)
            nc.vector.tensor_tensor(s01[:], psq[:, :, :, 0], psq[:, :, :, 1],
                                    OP.add)
            nc.vector.tensor_tensor(pts4[:, :, :, 3], s01[:],
                                    psq[:, :, :, 2], OP.add)

            # shared coefficient row (DMA'd into each bd's staged block)
            rc = zp.tile([1, 7, 512], fp32)
            nc.vector.memset(rc[:, 0:3, :], 1.0)
            nc.vector.memset(rc[:, 3:6, :], 2.0)
            nc.vector.memset(rc[:, 6:7, :], 0.0)

            # one-hot face matrices per hand s, corner k, K-chunk kk
            # + identity one-hot (for pts transposition via gather matmul)
            oh = zp.tile([128, 2, 3, 2, 512], fp32)
            idh = zp.tile([128, 2, PPAD], fp32)
            PT = zp.tile([3, PPAD], fp32)
            SQ = zp.tile([3, PPAD], fp32)
            one256 = zp.tile([1, PPAD], fp32)
            nc.vector.memset(one256[:], 1.0)
            with tc.tile_pool(name="ph0bc", bufs=1, space="PSUM") as bp:
                bc = bp.tile([128, 3, 512], fp32)
                bcid = bp.tile([128, PPAD], fp32)
                ptp = bp.tile([3, PPAD], fp32)
                sqp = bp.tile([1, PPAD], fp32)
                for s in range(2):
                    for k in range(3):
                        nc.tensor.matmul(bc[:, k, :], ones1h[:],
                                         faces_sb[:, s, k, :])
                    for k in range(3):
                        for kk in range(2):
                            nc.vector.tensor_scalar(
                                oh[:, s, k, kk, :], bc[:, k, :],
                                cst_sb[:, kk:kk + 1], None, OP.is_equal)
                nc.tensor.matmul(bcid[:], ones1[:], extra_sb[:])
                for kk in range(2):
                    nc.vector.tensor_scalar(idh[:, kk, :], bcid[:],
                                            cst_sb[:, kk:kk + 1], None,
                                            OP.is_equal)
                # lhsT rows from pts: -2*pts^T via identity-gather matmuls,
                # |p|^2 via square + ones3-reduce
                for bd in range(NBD):
                    nc.tensor.matmul(ptp[:], pts_sb[:, 0, bd, :],
                                     idh[:, 0, :], start=True, stop=False)
                    nc.tensor.matmul(ptp[:], pts_sb[:, 1, bd, :],
                                     idh[:, 1, :], start=False, stop=True)
                    nc.scalar.mul(lhsT_sb[0:3, bd, :], ptp[:], -2.0)
                    nc.scalar.activation(PT[:], ptp[:], AF.Copy)
                    nc.vector.tensor_tensor(SQ[:], PT[:], PT[:], OP.mult)
                    nc.tensor.matmul(sqp[:], ones3[:], SQ[:])
                    sq1 = zp.tile([1, PPAD], fp32, name="sq1", tag="sq1",
                                  bufs=2)
                    nc.scalar.activation(sq1[:], sqp[:], AF.Copy)
                    nc.sync.dma_start(lhsT_sb[4:5, bd, :], sq1[:])
                    nc.sync.dma_start(lhsT_sb[3:4, bd, :], one256[:])

            # mrhs: rows0..2 = -0.5*lhsT rows0..2 (= vert xyz),
            # row3 <- lhsT row4 (|v|^2), row4 <- lhsT row3 (ones)
            nc.vector.tensor_scalar(mrhs_sb[0:3], lhsT_sb[0:3], -0.5, None,
                                    OP.mult)
            nc.sync.dma_start(mrhs_sb[3:4], lhsT_sb[4:5])
            nc.sync.dma_start(mrhs_sb[4:5], lhsT_sb[3:4])

            E1 = zp.tile([3, 512], fp32)
            E2 = zp.tile([3, 512], fp32)
            rotc = zp.tile([3, 4, 512], fp32)
            t1 = zp.tile([3, 512], fp32)
            t2 = zp.tile([3, 512], fp32)

            with tc.tile_pool(name="ph0ps", bufs=1, space="PSUM") as pp0:
                crn = [pp0.tile([4, 512], fp32, name=f"crn{t}", tag=t)
                       for t in "abc"]
                rot = pp0.tile([3, 4, 512], fp32)
                c3p = pp0.tile([1, 512], fp32)
                for bd in range(NBD):
                    other = bd ^ 1
                    s = other & 1
                    for k in range(3):
                        nc.tensor.matmul(crn[k][:], pts4[:, 0, other, :],
                                         oh[:, s, k, 0, :],
                                         start=True, stop=False)
                        nc.tensor.matmul(crn[k][:], pts4[:, 1, other, :],
                                         oh[:, s, k, 1, :],
                                         start=False, stop=True)
                    # double-buffered so bd+1's build overlaps bd's tail.
                    # asm4[:, k, :] = [xyz, |corner|^2] of corner k (the
                    # full 4-row PSUM drain keeps the ACT read at
                    # partition base 0; row 3 then rides a plain SBUF DMA
                    # into the staged c3 row).  asm holds groups 3..6.
                    asm4 = zp.tile([4, 3, 512], fp32, name="asm4",
                                   tag="asm4", bufs=2)
                    asm = zp.tile([3, 4, 512], fp32, name="asm", tag="asm",
                                  bufs=2)
                    PRD = zp.tile([3, 4, 512], fp32, name="PRD", tag="PRD",
                                  bufs=2)
                    C3r = zp.tile([1, 4, 512], fp32, name="C3r", tag="C3r",
                                  bufs=2)
                    nc.scalar.activation(asm4[:, 0, :], crn[0][:], AF.Copy)
                    nc.scalar.activation(asm4[:, 1, :], crn[1][:], AF.Copy)
                    nc.scalar.activation(asm4[:, 2, :], crn[2][:], AF.Copy)
                    Ac = asm4[0:3, 0, :]
                    Bc = asm4[0:3, 1, :]
                    Cc = asm4[0:3, 2, :]
                    nc.vector.tensor_tensor(E1[:], Bc, Ac, OP.subtract)
                    nc.vector.tensor_tensor(E2[:], Cc, Ac, OP.subtract)
                    # n = E1 x E2 via rotations: rot1/rot2 = P1^T/P2^T
                    nc.tensor.matmul(rot[:, 0, :], cst_sb[0:3, 2:5], E1[:])
                    nc.tensor.matmul(rot[:, 1, :], cst_sb[0:3, 5:8], E2[:])
                    nc.tensor.matmul(rot[:, 2, :], cst_sb[0:3, 5:8], E1[:])
                    nc.tensor.matmul(rot[:, 3, :], cst_sb[0:3, 2:5], E2[:])
                    nc.scalar.activation(rotc[:], rot[:], AF.Copy)
                    nc.vector.tensor_tensor(t1[:], rotc[:, 0, :],
                                            rotc[:, 1, :], OP.mult)
                    nc.vector.tensor_tensor(t2[:], rotc[:, 2, :],
                                            rotc[:, 3, :], OP.mult)
                    nc.vector.tensor_tensor(asm[:, 3, :], t1[:], t2[:],
                                            OP.subtract)
                    # products for the remaining c3 reduces + midpoint sums
                    # (g0..2 = |A|^2.. rode the corner gathers)
                    nc.vector.tensor_tensor(PRD[:, 0, :], Ac, Bc, OP.mult)
                    nc.vector.tensor_tensor(PRD[:, 1, :], Bc, Cc, OP.mult)
                    nc.vector.tensor_tensor(PRD[:, 2, :], Cc, Ac, OP.mult)
                    nc.vector.tensor_tensor(PRD[:, 3, :], Ac, asm[:, 3, :],
                                            OP.mult)
                    nc.vector.tensor_tensor(asm[:, 0, :], Ac, Bc, OP.add)
                    nc.vector.tensor_tensor(asm[:, 1, :], Bc, Cc, OP.add)
                    nc.vector.tensor_tensor(asm[:, 2, :], Cc, Ac, OP.add)
                    for g in range(4):
                        nc.tensor.matmul(c3p[:], twos3[:], PRD[:, g, :])
                        nc.scalar.activation(C3r[:, g, :], c3p[:], AF.Copy)
                    # assemble this bd's staged block: xyz rows (corners
                    # from asm4, groups 3..6 from asm), c3 row (|corner|^2
                    # from asm4 row 3, rest from C3r), coeffs
                    nc.sync.dma_start(staged[5 * bd:5 * bd + 3, 0:3],
                                      asm4[0:3, :, :])
                    nc.sync.dma_start(staged[5 * bd:5 * bd + 3, 3:7],
                                      asm[:])
                    nc.sync.dma_start(staged[5 * bd + 3:5 * bd + 4, 0:3],
                                      asm4[3:4, :, :])
                    nc.sync.dma_start(staged[5 * bd + 3:5 * bd + 4, 3:7],
                                      C3r[:])
                    nc.sync.dma_start(staged[5 * bd + 4:5 * bd + 5], rc[:])

        # ---------------- phase 1: main compute loop --------------------
        with (
            tc.tile_pool(name="store", bufs=1) as spool,
            tc.tile_pool(name="stage", bufs=2) as stpool,
            tc.tile_pool(name="iface", bufs=2) as ipool,
            tc.tile_pool(name="dve", bufs=1) as vpool,
        ):
            denoms = spool.tile([128, SUPER, 512], fp32)
            tts = spool.tile([128, SUPER, 512], fp32)

            def pass_a(ppool, i, j):
                bd, ch = divmod(i, 2)
                if ch == 0:
                    fstage = stpool.tile([5, 7, 512], fp32, tag="fstage")
                    nc.sync.dma_start(fstage[:], staged[5 * bd:5 * bd + 5])
                    pass_a.stage = fstage
                fstage = pass_a.stage
                lhs = lhsT_sb[:, bd, ch * 128:(ch + 1) * 128]

                wind = ppool.tile([128, 7, 512], fp32, tag="wind")
                md = ppool.tile([128, 256], fp32, tag="md")

                # per-group matmuls pipeline with the ACT/DVE consumers:
                # group 0's results stream downstream while groups 1-6 are
                # still on the PE (a single merged [128, 7x512] matmul
                # simmed 61us SLOWER despite saving 6 fixed overheads)
                for g in range(7):
                    nc.tensor.matmul(wind[:, g, :F], lhs, fstage[:, g, :F])
                nc.tensor.matmul(md[:, :P], lhs, mrhs_sb[:, bd ^ 1, :P])

                # early PSUM drain: groups 3..5 are otherwise read only
                # mid-DVE-chain, holding the bufs=1 wind tile (14KB of the
                # 16KB PSUM) and stalling the next block's matmuls.  With
                # this copy + rl + dets all early ACT ops, PSUM frees
                # ~2.6us after the matmuls instead of ~6us later.
                windc3 = ipool.tile([128, 3, 512], fp32, tag="windc3")
                nc.scalar.activation(windc3[:, :, :F], wind[:, 3:6, :F],
                                     AF.Copy)

                # min-distance: free-dim min, clamp at 0 (matmul roundoff)
                mind = vpool.tile([128, 1], fp32, tag="mind")
                nc.vector.tensor_reduce(mind[:], md[:, :P], AX, OP.min)
                nc.vector.tensor_scalar(minda[:, i:i + 1], mind[:], 0.0, None,
                                        OP.max)

                # norms: clamp squared lengths at 0, sqrt (one multi-dim-AP
                # op per stage instead of three).  dets reads wind BEFORE
                # sq3 (which doesn't) so the PSUM hold ends early.
                rl = ipool.tile([128, 3, 512], fp32, tag="rl")
                nc.scalar.activation(rl[:, :, :F], wind[:, 0:3, :F], AF.Relu)
                dets = ipool.tile([128, 512], fp32, tag="dets")
                nc.scalar.mul(dets[:, :F], wind[:, 6, :F], 0.5)
                sq3 = ipool.tile([128, 3, 512], fp32, tag="sq3")
                nc.scalar.activation(sq3[:, :, :F], rl[:, :, :F], AF.Sqrt)
                la = sq3[:, 0, :]
                lb = sq3[:, 1, :]
                lc = sq3[:, 2, :]

                # denominator chain; wind groups 3..5 hold 2ab/2bc/2ca so
                # fold the x0.5 into the scalar_tensor_tensor ops
                u = vpool.tile([128, 512], fp32, tag="u")
                r4 = vpool.tile([128, 512], fp32, tag="r4")
                s5 = vpool.tile([128, 512], fp32, tag="s5")
                v = vpool.tile([128, 512], fp32, tag="v")
                w = vpool.tile([128, 512], fp32, tag="w")
                t6 = vpool.tile([128, 512], fp32, tag="t6")
                nc.vector.scalar_tensor_tensor(r4[:, :F], windc3[:, 1, :F],
                                               0.5, la[:, :F], OP.mult,
                                               OP.mult)
                nc.vector.scalar_tensor_tensor(s5[:, :F], windc3[:, 2, :F],
                                               0.5, lb[:, :F], OP.mult,
                                               OP.mult)
                nc.vector.tensor_tensor(u[:, :F], la[:, :F], lb[:, :F], OP.mult)
                nc.vector.scalar_tensor_tensor(v[:, :F], windc3[:, 0, :F],
                                               0.5, u[:, :F], OP.mult,
                                               OP.add)

                w_ = w[:, :F]
                nc.vector.tensor_tensor(w_, v[:, :F], lc[:, :F], OP.mult)
                nc.vector.tensor_tensor(t6[:, :F], r4[:, :F], s5[:, :F], OP.add)
                den = denoms[:, j, :F]
                nc.vector.tensor_tensor(den, w_, t6[:, :F], OP.add)

                # half-angle atan2 range reduction: tt = det / (rho + |den|)
                xx = ipool.tile([128, 512], fp32, tag="xx")
                yy = ipool.tile([128, 512], fp32, tag="yy")
                ss = vpool.tile([128, 512], fp32, tag="ss", bufs=2)
                rho = ipool.tile([128, 512], fp32, tag="rho")
                axd = ipool.tile([128, 512], fp32, tag="axd")
                dd = vpool.tile([128, 512], fp32, tag="dd")
                rd = vpool.tile([128, 512], fp32, tag="rd")
                nc.scalar.activation(xx[:, :F], den, AF.Square)
                nc.scalar.activation(yy[:, :F], dets[:, :F], AF.Square)
                nc.vector.scalar_tensor_tensor(ss[:, :F], xx[:, :F], 1e-20,
                                               yy[:, :F], OP.add, OP.add)
                nc.scalar.activation(rho[:, :F], ss[:, :F], AF.Sqrt)
                nc.scalar.activation(axd[:, :F], den, AF.Abs)
                nc.vector.tensor_tensor(dd[:, :F], rho[:, :F], axd[:, :F],
                                        OP.add)
                nc.vector.reciprocal_approx_fast(rd[:, :F], dd[:, :F])
                nc.vector.tensor_tensor(tts[:, j, :F], dets[:, :F], rd[:, :F],
                                        OP.mult)

            def pass_b(i, j):
                den = denoms[:, j, :F]
                tt = tts[:, j, :F]
                sgn = ipool.tile([128, 512], fp32, tag="sgn")
                spi = ipool.tile([128, 512], fp32, tag="spi")
                atn = ipool.tile([128, 512], fp32, tag="atn")
                c0 = vpool.tile([128, 512], fp32, tag="c0")
                c1 = vpool.tile([128, 512], fp32, tag="c1")
                sd = vpool.tile([128, 512], fp32, tag="sd")
                nc.scalar.activation(sgn[:, :F], tt, AF.Sign)
                nc.scalar.mul(spi[:, :F], sgn[:, :F], HALF_PI)
                nc.scalar.activation(atn[:, :F], tt, AF.Arctan)
                # half = atn + [den<0]*(pi/2*sign(det) - 2*atn)
                nc.vector.scalar_tensor_tensor(c0[:, :F], atn[:, :F], -2.0,
                                               spi[:, :F], OP.mult, OP.add)
                nc.vector.scalar_tensor_tensor(c1[:, :F], den, 0.0,
                                               c0[:, :F], OP.is_lt, OP.mult)
                nc.vector.scalar_tensor_tensor(sd[:, :F], atn[:, :F], 0.0,
                                               c1[:, :F], OP.add, OP.add,
                                               accum_out=sacc[:, i:i + 1])

            nc.vector.memset(sacc[:], 0.0)
            nc.vector.memset(minda[:], 1.0)
            with tc.tile_pool(name="psum", bufs=1, space="PSUM") as ppool:
                for sg in range(0 if SKIP_P1 else NBLK // SUPER):
                    for j in range(SUPER):
                        pass_a(ppool, sg * SUPER + j, j)
                    for j in range(SUPER):
                        pass_b(sg * SUPER + j, j)

            # ------------- final: depth * inside, partition-reduce -------
            inside = cpool.tile([128, NBLK], fp32)
            depth = cpool.tile([128, NBLK], fp32)
            contrib = cpool.tile([128, NBLK], fp32)
            nc.vector.tensor_scalar(inside[:], sacc[:], HALF_PI, None,
                                    OP.is_gt)
            nc.scalar.activation(depth[:], minda[:], AF.Sqrt, bias=beps[:])
            nc.vector.tensor_tensor(contrib[:], depth[:], inside[:], OP.mult)

            with tc.tile_pool(name="psum2", bufs=1, space="PSUM") as p2:
                lpsum = p2.tile([NBLK, 1], fp32)
                nc.tensor.matmul(lpsum[:], contrib[:], ones[:])
                loss_sb = cpool.tile([NBLK, 1], fp32)
                nc.scalar.activation(loss_sb[:], lpsum[:], AF.Copy)
                nc.sync.dma_start(loss_d[:], loss_sb[:])


def _build():
    global _compiled
    if _compiled is not None:
        return _compiled
    import concourse.bacc as bacc
    import concourse.mybir as mybir
    import concourse.tile as tile

    nc = bacc.Bacc("TRN2", target_bir_lowering=False, debug=False,
                   num_devices=NCORES)
    fp32 = mybir.dt.float32
    u8 = mybir.dt.uint8
    pkd_d = nc.dram_tensor("pkd", (128, 312), u8, kind="ExternalInput").ap()
    cst_d = nc.dram_tensor("cst", (128, 8), fp32, kind="ExternalInput").ap()
    extra_d = nc.dram_tensor("extra", (1, PPAD), fp32, kind="ExternalInput").ap()
    loss_d = nc.dram_tensor("loss", (NBLK, 1), fp32, kind="ExternalOutput").ap()

    with tile.TileContext(nc) as tc:
        _kernel_body(tc, pkd_d, cst_d, extra_d, loss_d)
    nc.compile()
    _compiled = nc
    return nc


# --------------------------------------------------------------------------
# cached jitted runner + entry point
# --------------------------------------------------------------------------

def _build_runner():
    global _runner
    if _runner is not None:
        return _runner
    import jax
    from jax.sharding import Mesh, PartitionSpec
    from jax.experimental.shard_map import shard_map
    import concourse.mybir as mybir
    from concourse.bass2jax import (_bass_exec_p, partition_id_tensor,
                                    install_neuronx_cc_hook)

    nc = _build()
    install_neuronx_cc_hook()
    pname = nc.partition_id_tensor.name if nc.partition_id_tensor else None
    in_names, out_names, out_avals, zero_outs = [], [], [], []
    for alloc in nc.m.functions[0].allocations:
        if not isinstance(alloc, mybir.MemoryLocationSet):
            continue
        name = alloc.memorylocations[0].name
        if alloc.kind == "ExternalInput":
            if name != pname:
                in_names.append(name)
        elif alloc.kind == "ExternalOutput":
            out_names.append(name)
            shape = tuple(alloc.tensor_shape)
            dtype = mybir.dt.np(alloc.dtype)
            out_avals.append(jax.core.ShapedArray(shape, dtype))
            zero_outs.append(np.zeros(shape, dtype))
    n_params, n_outs = len(in_names), len(out_avals)
    in_names_full = in_names + out_names + ([pname] if pname else [])

    def _body(*args):
        operands = list(args)
        if pname is not None:
            operands.append(partition_id_tensor())
        return tuple(_bass_exec_p.bind(
            *operands, out_avals=tuple(out_avals), in_names=tuple(in_names_full),
            out_names=tuple(out_names), lowering_input_output_aliases=(),
            sim_require_finite=True, sim_require_nnan=True, nc=nc))

    devices = jax.devices()[:NCORES]
    mesh = Mesh(np.asarray(devices), ("core",))
    in_specs = (PartitionSpec("core"),) * (n_params + n_outs)
    out_specs = (PartitionSpec("core"),) * len(out_names)
    # no donation: the kernel writes every output element, so uninit
    # result buffers are fine and the zero operands can stay device-resident
    # forever (their per-call 8-shard upload caused an intermittent ~40ms
    # slow tail)
    sharded = jax.jit(
        shard_map(_body, mesh=mesh, in_specs=in_specs, out_specs=out_specs,
                  check_rep=False),
        keep_unused=True)
    # cst/extra are pure algorithm constants (iota columns, permutation
    # matrices): commit them device-resident once; passing the committed
    # arrays skips their per-call host->device processing (~4ms).
    from jax.sharding import NamedSharding
    shc = NamedSharding(mesh, PartitionSpec("core"))
    resident = {"cst": jax.device_put(_cst_concat, shc),
                "extra": jax.device_put(_extra_concat, shc)}
    rzeros = [jax.device_put(
        np.zeros((NCORES * z.shape[0],) + z.shape[1:], z.dtype), shc)
        for z in zero_outs]
    jax.block_until_ready(list(resident.values()) + rzeros)
    _runner = (sharded, in_names, rzeros, resident, shc)
    return _runner


def kernel(**inputs) -> np.ndarray:
    global last_exec_time_ns
    sharded, in_names, rzeros, resident, shc = _build_runner()
    # single direct 8-way sharded upload: the dev0-then-reshard hop
    # measured 1.3ms SLOWER than direct sharding, and one packed command
    # beats three separate arrays by ~1ms of per-command overhead.
    by_name = {"pkd": _prep(inputs)}
    concat_in = [resident.get(nm, by_name.get(nm)) for nm in in_names]
    out = sharded(*concat_in, *rzeros)
    last_exec_time_ns = None

    # block i = (b_loc*2 + dir)*2 + chunk -> sum each batch's 4 blocks
    o0 = np.asarray(out[0]).reshape(NCORES, NB, 4)
    return np.ascontiguousarray(o0.sum(axis=2).reshape(B).astype(np.float32))



# revision 49
# speedup vs baseline: 1.1034x; 1.0068x over previous
"""Trainium2 Bass kernel for nn_HandIntersectionLoss.

Strategy
--------
Pure data parallel over batch: 64 batches -> 8 cores x 8 local batches.

Wall-clock per call is dominated by the axon tunnel, so the host ships
only the gathered hand points (u24-packed, ~40KB/core) and the device
derives all per-(batch,face) matmul constants itself:

  phase 0 (device):
    - one-hot face matrices from f32 face indices (K=1 broadcast matmul
      + is_equal against shipped iota columns)
    - triangle corners A,B,C per (batch,dir) via 2-chunk accumulated
      gather matmuls:  corners[3,500] = pts[128,3]^T @ onehot[128,500]
    - edges E1=B-A, E2=C-A; normal n = E1 x E2 via permutation-matmul
      rotations (engines cannot read partition offsets != 0)
    - dots |A|^2,.., 2A.B,.., 2A.n via ones/twos-column reduce matmuls
    - constants assembled into a persistent `staged` SBUF tile
      ([65,7,512]: 4 rows per (batch,dir) + shared coefficient row)
      via SBUF->SBUF DMAs (the only legal cross-partition mover)

  phase 1 (device): the proven compute loop. Per 128-point block:
    K=5 matmuls against staged constants produce la^2,lb^2,lc^2,
    2ab,2bc,2ca, 2det for [128 points x 500 faces]; per-element chain
    (denominator + range-reduced atan2) on DVE/ACT:

      atan2(det, den) = 2*atan(det / (rho + |den|))            (den >= 0)
                      = sign(det)*pi - 2*atan(det/(rho+|den|)) (den < 0)
      rho = sqrt(det^2 + den^2 + 1e-20)

    inside(p) <=> sum_f half > pi/2.  Min-distance via the same matmul
    trick against derived vert constants (mrhs) + free-dim min-reduce.
    Scalar-engine table sets force the two-pass structure (sqrt vs
    arctan live in different ACT table sets), staged in super-groups.

The jitted shard_map callable is cached across kernel() calls so repeat
calls skip jax retrace/XLA recompile entirely.

Wire format (wall clock = tunnel RTT ~40ms + payload at ~55-60MB/s, so
payload bytes rule everything): points ship as u24 fixed point over
[-8, 8) (abs step 9.5e-7; loss rel-err 5.2e-3, a plateau set by one
zero-margin inside/outside flip — f16 at 2.6e-2 breaches the 2e-2 gate,
u16 at 1.8e-2 has no margin, u21..u24 are all on the plateau).  Faces
ship as u8 (indices < 251).  Everything packs into ONE u8 array
[1024, 312] uploaded direct 8-way sharded ([128, 312] per core): one
command beats three by ~1ms, and direct sharding beats the old
dev0+reshard hop by ~1.3ms.  Byte planes are PLANAR (b0|b1|b2|faces),
which lets the relay's zstd shrink the low-entropy gaussian b2 plane:
319,488B raw -> ~240,500B on the wire when the compressor keeps up.
Device decodes pts = (b0 + b1*256 + b2*65536) * 16/(2^24-1) - 8 in
three engine ops.

Group semantics (raw, no halving on device):
  g0..2: xyz=A|B|C,       c3=|A|^2..,  w=1
  g3..5: xyz=(A+B)..raw,  c3=2A.B..,   w=2   -> col = 2*(A-p).(B-p)
  g6:    xyz=n raw,       c3=2*A.n,    w=0   -> col = 2*det
pass_a compensates with x0.5 folded into existing scalar_tensor_tensor.
"""
import sys
import numpy as np

sys.path.insert(0, '/opt/trn_rl_repo')

B, V_FULL, V_HAND, V_LOOP, N_FACES = 64, 6890, 250, 20, 500
P = V_HAND + 1          # 251 points/verts per hand (incl. lid)
PPAD = 256
NCORES = 8
NB = B // NCORES        # local batches per core
NBD = NB * 2            # (batch, dir) pairs per core
NBLK = NBD * 2          # blocks per core: x2 point-chunks of 128
SUPER = 8               # blocks per two-pass super-group
F = N_FACES
HALF_PI = float(np.pi / 2)

_compiled = None
SKIP_P1 = False
_runner = None
last_exec_time_ns = None

# u24 fixed point over [-8, 8): x -> round((x+8) * (2^24-1)/16)
Q_SCALE = (2.0 ** 24 - 1) / 16.0
DQ_SCALE = 16.0 / (2.0 ** 24 - 1)


# --------------------------------------------------------------------------
# host prep: index gathers + u24 encode (all heavy constant math on-device)
# --------------------------------------------------------------------------

# preallocated per-call buffers (pad columns written once; concat layouts
# built directly to skip per-core copies).  Pad points sit at 7.5 — far
# outside the unit-scale hand cloud (winding ~0) yet clear of the u24
# top end (8.0 would round to 2^24 in f32 and wrap to -8 in the byte
# split).
#
# Everything dynamic ships in ONE u8 array [1024, 312] (sharded to
# [128, 312] per core) — one command beats three by ~1ms of per-command
# tunnel overhead.  Byte PLANES are kept separate (planar, not
# interleaved triplets): the tunnel zstd-compresses the request, and the
# b2 plane (gaussian high byte, ~5.3 bits entropy) only compresses when
# contiguous.  Unit-stride planes also read faster on the DVE.
#   bytes [:,   0: 96] = b0 (low)   of u24 = round((x+8) * (2^24-1)/16)
#   bytes [:,  96:192] = b1 (mid)
#   bytes [:, 192:288] = b2 (high)
#   bytes [:, 288:312] = faces u8, row-major: buf[p, 288+j] = flat[p*24+j]
_pts_host = np.full((B, 2, PPAD, 3), 7.5, np.float32)
_ptsq = np.empty((NCORES, 128, 2, NBD, 3), np.float32)
_buf_concat = np.empty((NCORES * 128, 312), np.uint8)
_faces_concat = np.zeros((NCORES, 2, 3, 512), np.uint8)
_cst_concat = np.zeros((NCORES * 128, 8), np.float32)
for _c in range(NCORES):
    _cs = _cst_concat[_c * 128:(_c + 1) * 128]
    _cs[:, 0] = np.arange(128, dtype=np.float32)
    _cs[:, 1] = np.arange(128, 256, dtype=np.float32)
    for _m in range(3):
        _cs[(_m + 1) % 3, 2 + _m] = 1.0    # P1 (rot1)
        _cs[(_m + 2) % 3, 5 + _m] = 1.0    # P2 (rot2)
_extra_concat = np.ascontiguousarray(
    np.broadcast_to(np.arange(PPAD, dtype=np.float32), (NCORES, PPAD)))


def _prep(inputs):
    verts = np.asarray(inputs['verts_batch'], dtype=np.float32)
    hi = [np.asarray(inputs['hand_verts_inds_left']),
          np.asarray(inputs['hand_verts_inds_right'])]
    li = [np.asarray(inputs['hand_loop_verts_inds_left']),
          np.asarray(inputs['hand_loop_verts_inds_right'])]

    # pad stays 8.0 from init (pad cols never overwritten)
    for d in range(2):
        _pts_host[:, d, :V_HAND] = verts[:, hi[d]]
        _pts_host[:, d, V_HAND] = verts[:, li[d]].mean(axis=1,
                                                       dtype=np.float32)

    # [core, 128, 2kk, bd, 3] gather layout in one strided copy, then
    # u24 = trunc((x+8)*scale + 0.5) (round-half-up via the cast); clip
    # guards out-of-range inputs from wrapping in the byte split
    _ptsq[:] = _pts_host.reshape(NCORES, NBD, 2, 128, 3).transpose(
        0, 3, 2, 1, 4)
    np.multiply(_ptsq, Q_SCALE, out=_ptsq)
    np.add(_ptsq, 8.0 * Q_SCALE + 0.5, out=_ptsq)
    np.clip(_ptsq, 0.0, 2.0 ** 24 - 1, out=_ptsq)
    v = _ptsq.astype(np.uint32).reshape(NCORES * 128, 96)
    vb8 = v.view(np.uint8).reshape(NCORES * 128, 96, 4)
    _buf_concat[:, 0:96] = vb8[..., 0]
    _buf_concat[:, 96:192] = vb8[..., 1]
    _buf_concat[:, 192:288] = vb8[..., 2]

    fc = [np.asarray(inputs['hand_faces_left']),
          np.asarray(inputs['hand_faces_right'])]
    for s in range(2):
        _faces_concat[:, s, :, :F] = fc[s].T.astype(np.uint8)[None]
    _buf_concat.reshape(NCORES, 128, 312)[:, :, 288:] = \
        _faces_concat.reshape(NCORES, 128, 24)
    return _buf_concat


# --------------------------------------------------------------------------
# device kernel
# --------------------------------------------------------------------------

def _kernel_body(tc, pkd_d, cst_d, extra_d, loss_d):
    import concourse.mybir as mybir
    nc = tc.nc
    fp32 = mybir.dt.float32
    u8 = mybir.dt.uint8
    AF = mybir.ActivationFunctionType
    OP = mybir.AluOpType
    AX = mybir.AxisListType.X

    fp16 = mybir.dt.float16
    with tc.tile_pool(name="const", bufs=1) as cpool:
        lhsT_sb = cpool.tile([5, NBD, PPAD], fp32)
        mrhs_sb = cpool.tile([5, NBD, PPAD], fp32)
        staged = cpool.tile([80, 7, 512], fp32)
        ones = cpool.tile([128, 1], fp32)
        beps = cpool.tile([128, 1], fp32)
        sacc = cpool.tile([128, NBLK], fp32)
        minda = cpool.tile([128, NBLK], fp32)
        nc.vector.memset(ones[:], 1.0)
        nc.vector.memset(beps[:], 1e-12)

        # ---------------- phase 0: derive constants on device ----------
        with tc.tile_pool(name="ph0", bufs=1) as zp:
            ones1 = zp.tile([1, 128], fp32)
            ones1h = zp.tile([1, 128], fp16)
            ones3 = zp.tile([3, 1], fp32)
            twos3 = zp.tile([3, 1], fp32)
            nc.vector.memset(ones1[:], 1.0)
            nc.vector.memset(ones1h[:], 1.0)
            nc.vector.memset(ones3[:], 1.0)
            nc.vector.memset(twos3[:], 2.0)
            pts_sb = zp.tile([128, 2, NBD, 3], fp32)
            pkd_sb = zp.tile([128, 3, 96], u8)
            faces_u8 = zp.tile([1, 2, 3, 512], u8)
            faces_sb = zp.tile([1, 2, 3, 512], fp16)
            cst_sb = zp.tile([128, 8], fp32)
            extra_sb = zp.tile([1, PPAD], fp32)
            nc.sync.dma_start(pkd_sb[:], pkd_d[:, 0:288])
            nc.sync.dma_start(faces_u8[:], pkd_d[:, 288:312])
            nc.sync.dma_start(cst_sb[:], cst_d[:])
            nc.sync.dma_start(extra_sb[:], extra_d[:])
            # decode planar u24: pts = (b0 + b1*256 + b2*65536)*DQ - 8
            ptmp = zp.tile([128, 2, NBD, 3], fp32)
            ptmp2 = zp.tile([128, 2, NBD, 3], fp32)
            nc.vector.scalar_tensor_tensor(ptmp[:], pkd_sb[:, 1:2, :], 256.0,
                                           pkd_sb[:, 0:1, :], OP.mult, OP.add)
            nc.vector.scalar_tensor_tensor(ptmp2[:], pkd_sb[:, 2:3, :],
                                           65536.0, ptmp[:], OP.mult, OP.add)
            nc.scalar.activation(pts_sb[:], ptmp2[:], AF.Copy, bias=-8.0,
                                 scale=DQ_SCALE)
            # faces u8 -> f16 for the broadcast matmuls
            nc.vector.tensor_scalar(faces_sb[:], faces_u8[:], 0.0, None,
                                    OP.add)
            # pts4 = [xyz, |p|^2] per point: extending the corner-gather
            # lhsT to 4 rows makes |A|^2,|B|^2,|C|^2 fall out of the same
            # gather matmuls, killing 3 of the 7 c3 column-sum matmuls
            # per bd (~80us of quarter-rate f32 PE time)
            pts4 = zp.tile([128, 2, NBD, 4], fp32)
            psq = zp.tile([128, 2, NBD, 3], fp32)
            s01 = zp.tile([128, 2, NBD], fp32)
            nc.scalar.activation(pts4[:, :, :, 0:3], pts_sb[:], AF.Copy)
            nc.vector.tensor_tensor(psq[:], pts_sb[:], pts_sb[:], OP.mult)
            nc.vector.tensor_tensor(s01[:], psq[:, :, :, 0], psq[:, :, :, 1],
                                    OP.add)
            nc.vector.tensor_tensor(pts4[:, :, :, 3], s01[:],
                                    psq[:, :, :, 2], OP.add)

            # shared coefficient row (DMA'd into each bd's staged block)
            rc = zp.tile([1, 7, 512], fp32)
            nc.vector.memset(rc[:, 0:3, :], 1.0)
            nc.vector.memset(rc[:, 3:6, :], 2.0)
            nc.vector.memset(rc[:, 6:7, :], 0.0)

            # one-hot face matrices per hand s, corner k, K-chunk kk
            # + identity one-hot (for pts transposition via gather matmul)
            oh = zp.tile([128, 2, 3, 2, 512], fp32)
            idh = zp.tile([128, 2, PPAD], fp32)
            PT = zp.tile([3, PPAD], fp32)
            SQ = zp.tile([3, PPAD], fp32)
            one256 = zp.tile([1, PPAD], fp32)
            nc.vector.memset(one256[:], 1.0)
            with tc.tile_pool(name="ph0bc", bufs=1, space="PSUM") as bp:
                bc = bp.tile([128, 3, 512], fp32)
                bcid = bp.tile([128, PPAD], fp32)
                ptp = bp.tile([3, PPAD], fp32)
                sqp = bp.tile([1, PPAD], fp32)
                for s in range(2):
                    for k in range(3):
                        nc.tensor.matmul(bc[:, k, :], ones1h[:],
                                         faces_sb[:, s, k, :])
                    for k in range(3):
                        for kk in range(2):
                            nc.vector.tensor_scalar(
                                oh[:, s, k, kk, :], bc[:, k, :],
                                cst_sb[:, kk:kk + 1], None, OP.is_equal)
                nc.tensor.matmul(bcid[:], ones1[:], extra_sb[:])
                for kk in range(2):
                    nc.vector.tensor_scalar(idh[:, kk, :], bcid[:],
                                            cst_sb[:, kk:kk + 1], None,
                                            OP.is_equal)
                # lhsT rows from pts: -2*pts^T via identity-gather matmuls,
                # |p|^2 via square + ones3-reduce
                for bd in range(NBD):
                    nc.tensor.matmul(ptp[:], pts_sb[:, 0, bd, :],
                                     idh[:, 0, :], start=True, stop=False)
                    nc.tensor.matmul(ptp[:], pts_sb[:, 1, bd, :],
                                     idh[:, 1, :], start=False, stop=True)
                    nc.scalar.mul(lhsT_sb[0:3, bd, :], ptp[:], -2.0)
                    nc.scalar.activation(PT[:], ptp[:], AF.Copy)
                    nc.vector.tensor_tensor(SQ[:], PT[:], PT[:], OP.mult)
                    nc.tensor.matmul(sqp[:], ones3[:], SQ[:])
                    sq1 = zp.tile([1, PPAD], fp32, name="sq1", tag="sq1",
                                  bufs=2)
                    nc.scalar.activation(sq1[:], sqp[:], AF.Copy)
                    nc.sync.dma_start(lhsT_sb[4:5, bd, :], sq1[:])
                    nc.sync.dma_start(lhsT_sb[3:4, bd, :], one256[:])

            # mrhs: rows0..2 = -0.5*lhsT rows0..2 (= vert xyz),
            # row3 <- lhsT row4 (|v|^2), row4 <- lhsT row3 (ones)
            nc.vector.tensor_scalar(mrhs_sb[0:3], lhsT_sb[0:3], -0.5, None,
                                    OP.mult)
            nc.sync.dma_start(mrhs_sb[3:4], lhsT_sb[4:5])
            nc.sync.dma_start(mrhs_sb[4:5], lhsT_sb[3:4])

            E1 = zp.tile([3, 512], fp32)
            E2 = zp.tile([3, 512], fp32)
            rotc = zp.tile([3, 4, 512], fp32)
            t1 = zp.tile([3, 512], fp32)
            t2 = zp.tile([3, 512], fp32)

            with tc.tile_pool(name="ph0ps", bufs=1, space="PSUM") as pp0:
                crn = [pp0.tile([4, 512], fp32, name=f"crn{t}", tag=t)
                       for t in "abc"]
                rot = pp0.tile([3, 4, 512], fp32)
                c3p = pp0.tile([1, 512], fp32)
                for bd in range(NBD):
                    other = bd ^ 1
                    s = other & 1
                    for k in range(3):
                        nc.tensor.matmul(crn[k][:], pts4[:, 0, other, :],
                                         oh[:, s, k, 0, :],
                                         start=True, stop=False)
                        nc.tensor.matmul(crn[k][:], pts4[:, 1, other, :],
                                         oh[:, s, k, 1, :],
                                         start=False, stop=True)
                    # double-buffered so bd+1's build overlaps bd's tail.
                    # asm4[:, k, :] = [xyz, |corner|^2] of corner k (the
                    # full 4-row PSUM drain keeps the ACT read at
                    # partition base 0; row 3 then rides a plain SBUF DMA
                    # into the staged c3 row).  asm holds groups 3..6.
                    asm4 = zp.tile([4, 3, 512], fp32, name="asm4",
                                   tag="asm4", bufs=2)
                    asm = zp.tile([3, 4, 512], fp32, name="asm", tag="asm",
                                  bufs=2)
                    PRD = zp.tile([3, 4, 512], fp32, name="PRD", tag="PRD",
                                  bufs=2)
                    C3r = zp.tile([1, 4, 512], fp32, name="C3r", tag="C3r",
                                  bufs=2)
                    nc.scalar.activation(asm4[:, 0, :], crn[0][:], AF.Copy)
                    nc.scalar.activation(asm4[:, 1, :], crn[1][:], AF.Copy)
                    nc.scalar.activation(asm4[:, 2, :], crn[2][:], AF.Copy)
                    Ac = asm4[0:3, 0, :]
                    Bc = asm4[0:3, 1, :]
                    Cc = asm4[0:3, 2, :]
                    nc.vector.tensor_tensor(E1[:], Bc, Ac, OP.subtract)
                    nc.vector.tensor_tensor(E2[:], Cc, Ac, OP.subtract)
                    # n = E1 x E2 via rotations: rot1/rot2 = P1^T/P2^T
                    nc.tensor.matmul(rot[:, 0, :], cst_sb[0:3, 2:5], E1[:])
                    nc.tensor.matmul(rot[:, 1, :], cst_sb[0:3, 5:8], E2[:])
                    nc.tensor.matmul(rot[:, 2, :], cst_sb[0:3, 5:8], E1[:])
                    nc.tensor.matmul(rot[:, 3, :], cst_sb[0:3, 2:5], E2[:])
                    nc.scalar.activation(rotc[:], rot[:], AF.Copy)
                    nc.vector.tensor_tensor(t1[:], rotc[:, 0, :],
                                            rotc[:, 1, :], OP.mult)
                    nc.vector.tensor_tensor(t2[:], rotc[:, 2, :],
                                            rotc[:, 3, :], OP.mult)
                    nc.vector.tensor_tensor(asm[:, 3, :], t1[:], t2[:],
                                            OP.subtract)
                    # products for the remaining c3 reduces + midpoint sums
                    # (g0..2 = |A|^2.. rode the corner gathers)
                    nc.vector.tensor_tensor(PRD[:, 0, :], Ac, Bc, OP.mult)
                    nc.vector.tensor_tensor(PRD[:, 1, :], Bc, Cc, OP.mult)
                    nc.vector.tensor_tensor(PRD[:, 2, :], Cc, Ac, OP.mult)
                    nc.vector.tensor_tensor(PRD[:, 3, :], Ac, asm[:, 3, :],
                                            OP.mult)
                    nc.vector.tensor_tensor(asm[:, 0, :], Ac, Bc, OP.add)
                    nc.vector.tensor_tensor(asm[:, 1, :], Bc, Cc, OP.add)
                    nc.vector.tensor_tensor(asm[:, 2, :], Cc, Ac, OP.add)
                    for g in range(4):
                        nc.tensor.matmul(c3p[:], twos3[:], PRD[:, g, :])
                        nc.scalar.activation(C3r[:, g, :], c3p[:], AF.Copy)
                    # assemble this bd's staged block: xyz rows (corners
                    # from asm4, groups 3..6 from asm), c3 row (|corner|^2
                    # from asm4 row 3, rest from C3r), coeffs
                    nc.sync.dma_start(staged[5 * bd:5 * bd + 3, 0:3],
                                      asm4[0:3, :, :])
                    nc.sync.dma_start(staged[5 * bd:5 * bd + 3, 3:7],
                                      asm[:])
                    nc.sync.dma_start(staged[5 * bd + 3:5 * bd + 4, 0:3],
                                      asm4[3:4, :, :])
                    nc.sync.dma_start(staged[5 * bd + 3:5 * bd + 4, 3:7],
                                      C3r[:])
                    nc.sync.dma_start(staged[5 * bd + 4:5 * bd + 5], rc[:])

        # ---------------- phase 1: main compute loop --------------------
        with (
            tc.tile_pool(name="store", bufs=1) as spool,
            tc.tile_pool(name="stage", bufs=2) as stpool,
            tc.tile_pool(name="iface", bufs=2) as ipool,
            tc.tile_pool(name="dve", bufs=1) as vpool,
        ):
            denoms = spool.tile([128, SUPER, 512], fp32)
            tts = spool.tile([128, SUPER, 512], fp32)

            def pass_a(ppool, i, j):
                bd, ch = divmod(i, 2)
                if ch == 0:
                    fstage = stpool.tile([5, 7, 512], fp32, tag="fstage")
                    nc.sync.dma_start(fstage[:], staged[5 * bd:5 * bd + 5])
                    pass_a.stage = fstage
                fstage = pass_a.stage
                lhs = lhsT_sb[:, bd, ch * 128:(ch + 1) * 128]

                wind = ppool.tile([128, 7, 512], fp32, tag="wind")
                md = ppool.tile([128, 256], fp32, tag="md")

                # per-group matmuls pipeline with the ACT/DVE consumers:
                # group 0's results stream downstream while groups 1-6 are
                # still on the PE (a single merged [128, 7x512] matmul
                # simmed 61us SLOWER despite saving 6 fixed overheads)
                for g in range(7):
                    nc.tensor.matmul(wind[:, g, :F], lhs, fstage[:, g, :F])
                nc.tensor.matmul(md[:, :P], lhs, mrhs_sb[:, bd ^ 1, :P])

                # early PSUM drain: groups 3..5 are otherwise read only
                # mid-DVE-chain, holding the bufs=1 wind tile (14KB of the
                # 16KB PSUM) and stalling the next block's matmuls.  With
                # this copy + rl + dets all early ACT ops, PSUM frees
                # ~2.6us after the matmuls instead of ~6us later.
                windc3 = ipool.tile([128, 3, 512], fp32, tag="windc3")
                nc.scalar.activation(windc3[:, :, :F], wind[:, 3:6, :F],
                                     AF.Copy)

                # min-distance: free-dim min, clamp at 0 (matmul roundoff)
                mind = vpool.tile([128, 1], fp32, tag="mind")
                nc.vector.tensor_reduce(mind[:], md[:, :P], AX, OP.min)
                nc.vector.tensor_scalar(minda[:, i:i + 1], mind[:], 0.0, None,
                                        OP.max)

                # norms: clamp squared lengths at 0, sqrt (one multi-dim-AP
                # op per stage instead of three).  dets reads wind BEFORE
                # sq3 (which doesn't) so the PSUM hold ends early.
                rl = ipool.tile([128, 3, 512], fp32, tag="rl")
                nc.scalar.activation(rl[:, :, :F], wind[:, 0:3, :F], AF.Relu)
                dets = ipool.tile([128, 512], fp32, tag="dets")
                nc.scalar.mul(dets[:, :F], wind[:, 6, :F], 0.5)
                sq3 = ipool.tile([128, 3, 512], fp32, tag="sq3")
                nc.scalar.activation(sq3[:, :, :F], rl[:, :, :F], AF.Sqrt)
                la = sq3[:, 0, :]
                lb = sq3[:, 1, :]
                lc = sq3[:, 2, :]

                # denominator chain; wind groups 3..5 hold 2ab/2bc/2ca so
                # fold the x0.5 into the scalar_tensor_tensor ops
                u = vpool.tile([128, 512], fp32, tag="u")
                r4 = vpool.tile([128, 512], fp32, tag="r4")
                s5 = vpool.tile([128, 512], fp32, tag="s5")
                v = vpool.tile([128, 512], fp32, tag="v")
                w = vpool.tile([128, 512], fp32, tag="w")
                t6 = vpool.tile([128, 512], fp32, tag="t6")
                nc.vector.scalar_tensor_tensor(r4[:, :F], windc3[:, 1, :F],
                                               0.5, la[:, :F], OP.mult,
                                               OP.mult)
                nc.vector.scalar_tensor_tensor(s5[:, :F], windc3[:, 2, :F],
                                               0.5, lb[:, :F], OP.mult,
                                               OP.mult)
                nc.vector.tensor_tensor(u[:, :F], la[:, :F], lb[:, :F], OP.mult)
                nc.vector.scalar_tensor_tensor(v[:, :F], windc3[:, 0, :F],
                                               0.5, u[:, :F], OP.mult,
                                               OP.add)

                w_ = w[:, :F]
                nc.vector.tensor_tensor(w_, v[:, :F], lc[:, :F], OP.mult)
                nc.vector.tensor_tensor(t6[:, :F], r4[:, :F], s5[:, :F], OP.add)
                den = denoms[:, j, :F]
                nc.vector.tensor_tensor(den, w_, t6[:, :F], OP.add)

                # half-angle atan2 range reduction: tt = det / (rho + |den|)
                xx = ipool.tile([128, 512], fp32, tag="xx")
                yy = ipool.tile([128, 512], fp32, tag="yy")
                ss = vpool.tile([128, 512], fp32, tag="ss", bufs=2)
                rho = ipool.tile([128, 512], fp32, tag="rho")
                axd = ipool.tile([128, 512], fp32, tag="axd")
                dd = vpool.tile([128, 512], fp32, tag="dd")
                rd = vpool.tile([128, 512], fp32, tag="rd")
                nc.scalar.activation(xx[:, :F], den, AF.Square)
                nc.scalar.activation(yy[:, :F], dets[:, :F], AF.Square)
                nc.vector.scalar_tensor_tensor(ss[:, :F], xx[:, :F], 1e-20,
                                               yy[:, :F], OP.add, OP.add)
                nc.scalar.activation(rho[:, :F], ss[:, :F], AF.Sqrt)
                nc.scalar.activation(axd[:, :F], den, AF.Abs)
                nc.vector.tensor_tensor(dd[:, :F], rho[:, :F], axd[:, :F],
                                        OP.add)
                nc.vector.reciprocal_approx_fast(rd[:, :F], dd[:, :F])
                nc.vector.tensor_tensor(tts[:, j, :F], dets[:, :F], rd[:, :F],
                                        OP.mult)

            def pass_b(i, j):
                den = denoms[:, j, :F]
                tt = tts[:, j, :F]
                sgn = ipool.tile([128, 512], fp32, tag="sgn")
                spi = ipool.tile([128, 512], fp32, tag="spi")
                atn = ipool.tile([128, 512], fp32, tag="atn")
                c0 = vpool.tile([128, 512], fp32, tag="c0")
                c1 = vpool.tile([128, 512], fp32, tag="c1")
                sd = vpool.tile([128, 512], fp32, tag="sd")
                nc.scalar.activation(sgn[:, :F], tt, AF.Sign)
                nc.scalar.mul(spi[:, :F], sgn[:, :F], HALF_PI)
                nc.scalar.activation(atn[:, :F], tt, AF.Arctan)
                # half = atn + [den<0]*(pi/2*sign(det) - 2*atn)
                nc.vector.scalar_tensor_tensor(c0[:, :F], atn[:, :F], -2.0,
                                               spi[:, :F], OP.mult, OP.add)
                nc.vector.scalar_tensor_tensor(c1[:, :F], den, 0.0,
                                               c0[:, :F], OP.is_lt, OP.mult)
                nc.vector.scalar_tensor_tensor(sd[:, :F], atn[:, :F], 0.0,
                                               c1[:, :F], OP.add, OP.add,
                                               accum_out=sacc[:, i:i + 1])

            nc.vector.memset(sacc[:], 0.0)
            nc.vector.memset(minda[:], 1.0)
            with tc.tile_pool(name="psum", bufs=1, space="PSUM") as ppool:
                for sg in range(0 if SKIP_P1 else NBLK // SUPER):
                    for j in range(SUPER):
                        pass_a(ppool, sg * SUPER + j, j)
                    for j in range(SUPER):
                        pass_b(sg * SUPER + j, j)

            # ------------- final: depth * inside, partition-reduce -------
            inside = cpool.tile([128, NBLK], fp32)
            depth = cpool.tile([128, NBLK], fp32)
            contrib = cpool.tile([128, NBLK], fp32)
            nc.vector.tensor_scalar(inside[:], sacc[:], HALF_PI, None,
                                    OP.is_gt)
            nc.scalar.activation(depth[:], minda[:], AF.Sqrt, bias=beps[:])
            nc.vector.tensor_tensor(contrib[:], depth[:], inside[:], OP.mult)

            with tc.tile_pool(name="psum2", bufs=1, space="PSUM") as p2:
                lpsum = p2.tile([NBLK, 1], fp32)
                nc.tensor.matmul(lpsum[:], contrib[:], ones[:])
                loss_sb = cpool.tile([NBLK, 1], fp32)
                nc.scalar.activation(loss_sb[:], lpsum[:], AF.Copy)
                nc.sync.dma_start(loss_d[:], loss_sb[:])


def _build():
    global _compiled
    if _compiled is not None:
        return _compiled
    import concourse.bacc as bacc
    import concourse.mybir as mybir
    import concourse.tile as tile

    nc = bacc.Bacc("TRN2", target_bir_lowering=False, debug=False,
                   num_devices=NCORES)
    fp32 = mybir.dt.float32
    u8 = mybir.dt.uint8
    pkd_d = nc.dram_tensor("pkd", (128, 312), u8, kind="ExternalInput").ap()
    cst_d = nc.dram_tensor("cst", (128, 8), fp32, kind="ExternalInput").ap()
    extra_d = nc.dram_tensor("extra", (1, PPAD), fp32, kind="ExternalInput").ap()
    loss_d = nc.dram_tensor("loss", (NBLK, 1), fp32, kind="ExternalOutput").ap()

    with tile.TileContext(nc) as tc:
        _kernel_body(tc, pkd_d, cst_d, extra_d, loss_d)
    nc.compile()
    _compiled = nc
    return nc


# --------------------------------------------------------------------------
# cached jitted runner + entry point
# --------------------------------------------------------------------------

def _build_runner():
    global _runner
    if _runner is not None:
        return _runner
    import jax
    from jax.sharding import Mesh, PartitionSpec
    from jax.experimental.shard_map import shard_map
    import concourse.mybir as mybir
    from concourse.bass2jax import (_bass_exec_p, partition_id_tensor,
                                    install_neuronx_cc_hook)

    nc = _build()
    install_neuronx_cc_hook()
    pname = nc.partition_id_tensor.name if nc.partition_id_tensor else None
    in_names, out_names, out_avals, zero_outs = [], [], [], []
    for alloc in nc.m.functions[0].allocations:
        if not isinstance(alloc, mybir.MemoryLocationSet):
            continue
        name = alloc.memorylocations[0].name
        if alloc.kind == "ExternalInput":
            if name != pname:
                in_names.append(name)
        elif alloc.kind == "ExternalOutput":
            out_names.append(name)
            shape = tuple(alloc.tensor_shape)
            dtype = mybir.dt.np(alloc.dtype)
            out_avals.append(jax.core.ShapedArray(shape, dtype))
            zero_outs.append(np.zeros(shape, dtype))
    n_params, n_outs = len(in_names), len(out_avals)
    in_names_full = in_names + out_names + ([pname] if pname else [])

    def _body(*args):
        operands = list(args)
        if pname is not None:
            operands.append(partition_id_tensor())
        return tuple(_bass_exec_p.bind(
            *operands, out_avals=tuple(out_avals), in_names=tuple(in_names_full),
            out_names=tuple(out_names), lowering_input_output_aliases=(),
            sim_require_finite=True, sim_require_nnan=True, nc=nc))

    devices = jax.devices()[:NCORES]
    mesh = Mesh(np.asarray(devices), ("core",))
    in_specs = (PartitionSpec("core"),) * (n_params + n_outs)
    out_specs = (PartitionSpec("core"),) * len(out_names)
    # no donation: the kernel writes every output element, so uninit
    # result buffers are fine and the zero operands can stay device-resident
    # forever (their per-call 8-shard upload caused an intermittent ~40ms
    # slow tail)
    sharded = jax.jit(
        shard_map(_body, mesh=mesh, in_specs=in_specs, out_specs=out_specs,
                  check_rep=False),
        keep_unused=True)
    # cst/extra are pure algorithm constants (iota columns, permutation
    # matrices): commit them device-resident once; passing the committed
    # arrays skips their per-call host->device processing (~4ms).
    from jax.sharding import NamedSharding
    shc = NamedSharding(mesh, PartitionSpec("core"))
    resident = {"cst": jax.device_put(_cst_concat, shc),
                "extra": jax.device_put(_extra_concat, shc)}
    rzeros = [jax.device_put(
        np.zeros((NCORES * z.shape[0],) + z.shape[1:], z.dtype), shc)
        for z in zero_outs]
    jax.block_until_ready(list(resident.values()) + rzeros)
    _runner = (sharded, in_names, rzeros, resident, shc)
    return _runner


def kernel(**inputs) -> np.ndarray:
    global last_exec_time_ns
    sharded, in_names, rzeros, resident, shc = _build_runner()
    # single direct 8-way sharded upload: the dev0-then-reshard hop
    # measured 1.3ms SLOWER than direct sharding, and one packed command
    # beats three separate arrays by ~1ms of per-command overhead.
    by_name = {"pkd": _prep(inputs)}
    concat_in = [resident.get(nm, by_name.get(nm)) for nm in in_names]
    out = sharded(*concat_in, *rzeros)
    last_exec_time_ns = None

    # block i = (b_loc*2 + dir)*2 + chunk -> sum each batch's 4 blocks
    o0 = np.asarray(out[0]).reshape(NCORES, NB, 4)
    return np.ascontiguousarray(o0.sum(axis=2).reshape(B).astype(np.float32))

